# revision 9
# baseline (speedup 1.0000x reference)
"""Multi-modality double-value attention on 8 TRN2 NeuronCores.

Sharding: data-parallel over batch (16 items -> 2 per core). Each core runs
the full attention block for its 2 items; weights are replicated. No
collectives. Host pre-transposes x to x^T and casts inputs to bf16; compute
is bf16 with fp32 PSUM accumulation; output is fp32.

v2: PE-array tiling + software pipelining.
 - scores: 2-way row tiling (64x128 mode) - both heads of a pair run
   concurrently on disjoint PE row groups (K=64 each, no zero padding).
 - AV + softmax denominators: 4-way column tiling (128x32 mode) - the two
   heads' value matmuls (2 x M=32 each) and 4 ones-column denominator
   accumulators share the array.
 - exp: one Scalar-engine activation per (pair, key-chunk) covering both
   heads' scores (reads 4 PSUM banks in a single [ksz, 1930] sweep).
 - key chunks are uniform 113/114 so every matmul keeps the same tile size
   (round_up -> 128) - no PE mode changes from ragged tails.
 - item 1's projections are emitted as filler work inside item 0's
   scalar-bound attention loop; item 0's output projection fills item 1's.
"""

import numpy as np
import ml_dtypes

B, N, C = 16, 906, 768
H = 12
D = 64
M1 = 513
N_CORES = 8
BPC = B // N_CORES          # batch items per core
KC = C // 128               # 6 contraction chunks over C
NPAIR = H // 2              # 6 head pairs
NCH = 8                     # key chunks (uniform 113/114)
_kszs = [114, 114] + [113] * 6
_kst = [sum(_kszs[:i]) for i in range(NCH)]
KCH = list(zip(_kst, _kszs))            # key chunks
MIXC = 4                                 # chunk containing key M1-1=512
MIXR = 512 - _kst[MIXC] + 1              # rows [0,MIXR) of chunk 4 are keys <= 512
QCH = KCH                                # query row chunks for out-proj
QP = [(0, 512), (512, N - 512)]          # query column blocks (A, B)
CPASS = [(0, 512), (512, C - 512)]       # column passes over C
SCALE = D ** -0.5

TRACE = False          # set by test.py to capture a HW profile
LAST_RESULTS = None    # BassKernelResults of the most recent run

_BUILT = None


def _install_trace_shim():
    """The image's antenv lacks axon_hooks; recreate it so trace=True works."""
    import sys, types
    if "antenv.axon_hooks" in sys.modules:
        return
    mod = types.ModuleType("antenv.axon_hooks")
    mod._hook = None
    mod.set_axon_ntff_profile_hook = lambda h: setattr(mod, "_hook", h)
    mod.get_axon_ntff_profile_hook = lambda: mod._hook
    sys.modules["antenv.axon_hooks"] = mod
    import antenv
    antenv.axon_hooks = mod
    from trn_agent_boot.trn_boot import _ntff_profile_via_ctypes
    mod.set_axon_ntff_profile_hook(_ntff_profile_via_ctypes("/opt/axon/libaxon_pjrt.so"))


def _build():
    import concourse.tile as tile
    from concourse import bacc, mybir

    BF = mybir.dt.bfloat16
    F32 = mybir.dt.float32
    AF = mybir.ActivationFunctionType

    nc = bacc.Bacc("TRN2", target_bir_lowering=False, debug=False, num_devices=N_CORES)

    xT_d = nc.dram_tensor("xT", [BPC, C, N], BF, kind="ExternalInput").ap()
    w_d = {
        wn: nc.dram_tensor(wn, [C, C], BF, kind="ExternalInput").ap()
        for wn in ("wq", "wk", "wv", "wvc", "wp")
    }
    bias_d = nc.dram_tensor("bias", [128, C], F32, kind="ExternalInput").ap()
    out_d = nc.dram_tensor("out", [BPC, N, C], F32, kind="ExternalOutput").ap()

    with tile.TileContext(nc) as tc:
        from contextlib import ExitStack
        from concourse import library_config

        with ExitStack() as ctx:
            wpool = ctx.enter_context(tc.tile_pool(name="wpool", bufs=1))
            sb = ctx.enter_context(tc.tile_pool(name="sb", bufs=1))
            ps = ctx.enter_context(tc.tile_pool(name="ps", bufs=1, space="PSUM"))

            nc.gpsimd.load_library(library_config.attn)

            mm = nc.tensor.matmul
            pcopy = nc.vector.tensor_copy

            # ---- constants: weights + bias + ones column ----
            w_sb = {}

            def load_w(wn):
                tiles = []
                for kc in range(KC):
                    t = wpool.tile([128, C], BF, name=f"{wn}_{kc}", tag=f"{wn}_{kc}")
                    nc.sync.dma_start(t[:], w_d[wn][kc * 128:(kc + 1) * 128, :])
                    tiles.append(t)
                w_sb[wn] = tiles

            load_w("wq")
            load_w("wk")

            # x^T item 0 first (feeds the first projections); item 1 is DMA'd
            # after the item-0 projections are emitted (slot reuse via tag)
            xT = {}

            def load_xT(it):
                for kc in range(KC):
                    t = sb.tile([128, N], BF, name=f"xT_{it}_{kc}", tag="xT", bufs=8)
                    nc.sync.dma_start(t[:], xT_d[it, kc * 128:(kc + 1) * 128, :])
                    xT[(it, kc)] = t

            load_xT(0)
            load_w("wv")
            load_w("wvc")
            load_w("wp")
            bias_sb = wpool.tile([128, C], F32, name="bias_sb", tag="bias_sb")
            nc.sync.dma_start(bias_sb[:], bias_d[:])
            ones_sb = wpool.tile([128, 1], BF, name="ones_sb", tag="ones_sb")
            nc.vector.memset(ones_sb[:, :], 1.0)

            qT, kT, vt, vct, mix, oT = {}, {}, {}, {}, {}, {}

            # ---- projection emitters (each is ~1 psum-bank of PE work) ----
            def qk_block(it, p, wn, blk):
                dst_map = qT if wn == "wq" else kT
                if blk == 0:
                    dst_map[(it, p)] = sb.tile(
                        [128, N], BF, name=f"{wn[1]}T_{it}_{p}",
                        tag="qT" if wn == "wq" else "kT", bufs=7)
                dst = dst_map[(it, p)]
                qs, qw = QP[blk]
                pp = ps.tile([128, 512], F32, name="pp", tag="proj", bufs=1)
                for kc in range(KC):
                    mm(pp[:, 0:qw],
                       lhsT=w_sb[wn][kc][:, p * 128:(p + 1) * 128],
                       rhs=xT[(it, kc)][:, qs:qs + qw],
                       start=(kc == 0), stop=(kc == KC - 1))
                pcopy(dst[:, qs:qs + qw], pp[:, 0:qw])

            def v_block(it, c, wn, blk):
                dst_map = vt if wn == "wv" else vct
                ts, tsz = KCH[c]
                if blk == 0:
                    dst = sb.tile([128, C], BF, name=f"{wn[1:]}_{it}_{c}",
                                  tag="v" if wn == "wv" else "vc", bufs=16)
                    # AV stationary loads may touch all 128 partitions; keep
                    # the unwritten tail rows finite
                    nc.vector.memset(dst[96:128, :], 0.0)
                    dst_map[(it, c)] = dst
                dst = dst_map[(it, c)]
                cs, cw = CPASS[blk]
                pp = ps.tile([128, 512], F32, name="pp", tag="proj", bufs=1)
                for kc in range(KC):
                    mm(pp[0:tsz, 0:cw],
                       lhsT=xT[(it, kc)][:, ts:ts + tsz],
                       rhs=w_sb[wn][kc][:, cs:cs + cw],
                       start=(kc == 0), stop=(kc == KC - 1))
                pcopy(dst[0:tsz, cs:cs + cw], pp[0:tsz, 0:cw])

            def mixes(it):
                # chunk MIXC (keys 454..566) straddles M1: rows [0,MIXR) are
                # keys <= 512 (modality a -> v), the rest modality v -> vc
                am = sb.tile([128, C], BF, name=f"amix_{it}", tag="amix", bufs=2)
                vm = sb.tile([128, C], BF, name=f"vmix_{it}", tag="vmix", bufs=2)
                pcopy(am[:, :], vct[(it, MIXC)][:, :])
                pcopy(am[0:MIXR, :], vt[(it, MIXC)][0:MIXR, :])
                pcopy(vm[:, :], vt[(it, MIXC)][:, :])
                pcopy(vm[0:MIXR, :], vct[(it, MIXC)][0:MIXR, :])
                mix[it] = (am, vm)

            def proj_fillers(it):
                fs = []
                for p in range(NPAIR):
                    for wn in ("wq", "wk"):
                        for blk in (0, 1):
                            fs.append(lambda it=it, p=p, wn=wn, blk=blk:
                                      qk_block(it, p, wn, blk))
                for c in (MIXC, 0, 1, 2, 3, 5, 6, 7):
                    for wn in ("wv", "wvc"):
                        for blk in (0, 1):
                            fs.append(lambda it=it, c=c, wn=wn, blk=blk:
                                      v_block(it, c, wn, blk))
                    if c == MIXC:
                        fs.append(lambda it=it: mixes(it))
                return fs

            def outproj_fillers(it):
                fs = []
                for c in range(NCH):
                    for blk in (0, 1):
                        def f(it=it, c=c, blk=blk):
                            ts, tsz = QCH[c]
                            cs, cw = CPASS[blk]
                            pp = ps.tile([128, 512], F32, name="pp", tag="proj", bufs=1)
                            for kp in range(NPAIR):
                                mm(pp[0:tsz, 0:cw],
                                   lhsT=oT[(it, kp)][:, ts:ts + tsz],
                                   rhs=w_sb["wp"][kp][:, cs:cs + cw],
                                   start=(kp == 0), stop=(kp == NPAIR - 1))
                            ob = sb.tile([128, 512], F32, name="ob", tag="ob", bufs=2)
                            nc.vector.tensor_add(ob[0:tsz, 0:cw], pp[0:tsz, 0:cw],
                                                 bias_sb[0:tsz, cs:cs + cw])
                            nc.sync.dma_start(out_d[it, ts:ts + tsz, cs:cs + cw],
                                              ob[0:tsz, 0:cw])
                        fs.append(f)
                return fs

            # ---- attention for one (item, pair); pops fillers each chunk ----
            def attn_pair(it, p, fillers):
                q_, k_ = qT[(it, p)], kT[(it, p)]
                # accumulators are pre-zeroed and every matmul uses
                # start=False: correct whether PSUM start-zeroing is
                # bank-wide or per-partition (semantics differ between the
                # simulator model and the HW docs)
                av = ps.tile([128, 1024], F32, name="av", tag="av", bufs=1)
                den = ps.tile([128, 512], F32, name="den", tag="den", bufs=1)
                nc.vector.memset(av[:, :], 0.0)
                nc.vector.memset(den[:, :], 0.0)

                def avden(c, ee, eo_):
                    # AV: 4-way col tiling (128x32). av rows 0:64 = head e,
                    # 64:128 = head o; cols 0:512 = queries 0:512 (mod a),
                    # col 512 = query 512 (mod a), cols 513:907 = queries
                    # 512:906 with mod-v values (col 513 is discarded).
                    # PSUM start=1 zero-arms a whole 2KB bank, so only the
                    # FIRST matmul touching each bank starts and only the
                    # LAST stops; interleaved column-tile groups rely on the
                    # per-byte lazy zeroing in between.
                    st, sp = (c == 0), (c == NCH - 1)
                    ksz = KCH[c][1]
                    va = mix[it][0] if c == MIXC else (vt[(it, c)] if c < MIXC else vct[(it, c)])
                    vv = mix[it][1] if c == MIXC else (vct[(it, c)] if c < MIXC else vt[(it, c)])
                    for ho, et in ((0, ee), (1, eo_)):
                        for dh in (0, 1):
                            m0 = ho * 64 + dh * 32
                            wc = p * 128 + m0
                            mm(av[m0:m0 + 32, 0:512], lhsT=va[0:ksz, wc:wc + 32],
                               rhs=et[0:ksz, 0:512], start=False, stop=sp,
                               tile_position=(0, m0), skip_group_check=True)
                            mm(av[m0:m0 + 32, 513:907], lhsT=vv[0:ksz, wc:wc + 32],
                               rhs=et[0:ksz, 512:906], start=False, stop=False,
                               tile_position=(0, m0), skip_group_check=True)
                            mm(av[m0:m0 + 32, 512:513], lhsT=va[0:ksz, wc:wc + 32],
                               rhs=et[0:ksz, 512:513], start=False, stop=sp,
                               tile_position=(0, m0), skip_group_check=True)
                    # denominators: 4 ones-column accumulators (rows 0/32/64/96)
                    mm(den[0:1, 0:512], lhsT=ones_sb[0:ksz, 0:1],
                       rhs=ee[0:ksz, 0:512], start=False, stop=sp,
                       tile_position=(0, 0), skip_group_check=True)
                    mm(den[32:33, 0:394], lhsT=ones_sb[0:ksz, 0:1],
                       rhs=ee[0:ksz, 512:906], start=False, stop=sp,
                       tile_position=(0, 32), skip_group_check=True)
                    mm(den[64:65, 0:512], lhsT=ones_sb[0:ksz, 0:1],
                       rhs=eo_[0:ksz, 0:512], start=False, stop=sp,
                       tile_position=(0, 64), skip_group_check=True)
                    mm(den[96:97, 0:394], lhsT=ones_sb[0:ksz, 0:1],
                       rhs=eo_[0:ksz, 512:906], start=False, stop=sp,
                       tile_position=(0, 96), skip_group_check=True)

                # chunk loop is software-pipelined one deep: AV/den for chunk
                # c-1 are emitted after the scores+exp of chunk c, so the PE
                # never sits behind the exp latency
                prev = None
                for c, (ks, ksz) in enumerate(KCH):
                    # scores: e/o heads row-tiled (64x128), separate bank pairs
                    sc = ps.tile([128, 2048], F32, name="sc", tag="sc", bufs=1)
                    for qs, qw in QP:
                        mm(sc[0:ksz, qs:qs + qw],
                           lhsT=k_[0:64, ks:ks + ksz], rhs=q_[0:64, qs:qs + qw],
                           start=True, stop=True)
                        mm(sc[0:ksz, 1024 + qs:1024 + qs + qw],
                           lhsT=k_[64:128, ks:ks + ksz], rhs=q_[64:128, qs:qs + qw],
                           start=True, stop=True)
                    # per-head exps: the e-head banks free while the o-head
                    # exp still runs, letting the next chunk's e-scores start
                    ee = sb.tile([128, 1024], BF, name="ee", tag="exp", bufs=6)
                    eo_ = sb.tile([128, 1024], BF, name="eo", tag="exp", bufs=6)
                    nc.scalar.activation(ee[0:ksz, 0:906], sc[0:ksz, 0:906],
                                         AF.Exp, scale=SCALE)
                    nc.scalar.activation(eo_[0:ksz, 0:906], sc[0:ksz, 1024:1930],
                                         AF.Exp, scale=SCALE)
                    if prev is not None:
                        avden(*prev)
                    prev = (c, ee, eo_)
                    if fillers:
                        fillers.pop(0)()
                avden(*prev)

                # drain the AV accumulator to SBUF so the next pair's AV can
                # start without waiting on this pair's softmax chain
                avf = sb.tile([128, 1024], F32, name="avf", tag="avf", bufs=2)
                pcopy(avf[:, 0:907], av[:, 0:907])

                # ---- softmax division ----
                rc = sb.tile([128, 512], F32, name="rc", tag="rc", bufs=1)
                nc.vector.reciprocal_approx_fast(rc[0:97, 0:512], den[0:97, 0:512])
                # partition_broadcast reads physical partition 0; relocate the
                # three off-zero reciprocal rows there first
                rl = sb.tile([128, 1300], F32, name="rl", tag="rl", bufs=1)
                nc.sync.dma_start(rl[0:1, 0:394], rc[32:33, 0:394])
                nc.sync.dma_start(rl[0:1, 394:906], rc[64:65, 0:512])
                nc.sync.dma_start(rl[0:1, 906:1300], rc[96:97, 0:394])
                bce = sb.tile([128, N], F32, name="bce", tag="bc", bufs=2)
                bco = sb.tile([128, N], F32, name="bco", tag="bc", bufs=2)
                nc.gpsimd.partition_broadcast(bce[:, 0:512], rc[0:1, 0:512])
                nc.gpsimd.partition_broadcast(bce[:, 512:906], rl[0:1, 0:394])
                nc.gpsimd.partition_broadcast(bco[:, 0:512], rl[0:1, 394:906])
                nc.gpsimd.partition_broadcast(bco[:, 512:906], rl[0:1, 906:1300])
                ot = sb.tile([128, N], BF, name=f"oT_{it}_{p}", tag="oT", bufs=8)
                for rows, bc in ((slice(0, 64), bce), (slice(64, 128), bco)):
                    nc.vector.tensor_mul(ot[rows, 0:513], avf[rows, 0:513],
                                         bc[rows, 0:513])
                    nc.vector.tensor_mul(ot[rows, 513:906], avf[rows, 514:907],
                                         bc[rows, 513:906])
                oT[(it, p)] = ot
                if fillers:
                    fillers.pop(0)()

            # ================= emission =================
            for p in range(NPAIR):
                for blk in (0, 1):
                    qk_block(0, p, "wq", blk)
                for blk in (0, 1):
                    qk_block(0, p, "wk", blk)
            for c in (MIXC, 0, 1, 2, 3, 5, 6, 7):
                for wn in ("wv", "wvc"):
                    for blk in (0, 1):
                        v_block(0, c, wn, blk)
                if c == MIXC:
                    mixes(0)

            load_xT(1)

            fill1 = proj_fillers(1)
            for p in range(NPAIR):
                attn_pair(0, p, fill1)
            for f in fill1:
                f()

            fill2 = outproj_fillers(0)
            for p in range(NPAIR):
                attn_pair(1, p, fill2)
            for f in fill2:
                f()
            for f in outproj_fillers(1):
                f()

    nc.compile()
    return nc


def _get_built():
    global _BUILT
    if _BUILT is None:
        _BUILT = _build()
    return _BUILT


def kernel(x, Wq, Wk, Wv, Wvc, Wp, bp):
    global LAST_RESULTS
    from concourse.bass_utils import run_bass_kernel_spmd

    x = np.asarray(x, dtype=np.float32)
    bf = ml_dtypes.bfloat16
    xT = np.ascontiguousarray(x.transpose(0, 2, 1)).astype(bf)      # (B, C, N)
    ws = {
        "wq": np.asarray(Wq, dtype=np.float32).astype(bf),
        "wk": np.asarray(Wk, dtype=np.float32).astype(bf),
        "wv": np.asarray(Wv, dtype=np.float32).astype(bf),
        "wvc": np.asarray(Wvc, dtype=np.float32).astype(bf),
        "wp": np.asarray(Wp, dtype=np.float32).astype(bf),
    }
    bias = np.ascontiguousarray(
        np.broadcast_to(np.asarray(bp, dtype=np.float32), (128, C))
    )

    if TRACE:
        _install_trace_shim()

    nc = _get_built()
    in_maps = []
    for i in range(N_CORES):
        m = {"xT": np.ascontiguousarray(xT[i * BPC:(i + 1) * BPC]), "bias": bias}
        m.update(ws)
        in_maps.append(m)

    res = run_bass_kernel_spmd(nc, in_maps, list(range(N_CORES)), trace=TRACE,
                               stitch_traces=False)
    LAST_RESULTS = res
    out = np.concatenate([res.results[i]["out"] for i in range(N_CORES)], axis=0)
    return out


# revision 11
# speedup vs baseline: 1.0619x; 1.0619x over previous
"""Multi-modality double-value attention on 8 TRN2 NeuronCores.

Sharding: data-parallel over batch (16 items -> 2 per core). Each core runs
the full attention block for its 2 items; weights are replicated. No
collectives. Host pre-transposes x to x^T and casts inputs to bf16; compute
is bf16 with fp32 PSUM accumulation; output is fp32.

v2: PE-array tiling + software pipelining.
 - scores: 2-way row tiling (64x128 mode) - both heads of a pair run
   concurrently on disjoint PE row groups (K=64 each, no zero padding).
 - AV + softmax denominators: 4-way column tiling (128x32 mode) - the two
   heads' value matmuls (2 x M=32 each) and 4 ones-column denominator
   accumulators share the array.
 - exp: one Scalar-engine activation per (pair, key-chunk) covering both
   heads' scores (reads 4 PSUM banks in a single [ksz, 1930] sweep).
 - key chunks are uniform 113/114 so every matmul keeps the same tile size
   (round_up -> 128) - no PE mode changes from ragged tails.
 - item 1's projections are emitted as filler work inside item 0's
   scalar-bound attention loop; item 0's output projection fills item 1's.
"""

import numpy as np
import ml_dtypes

B, N, C = 16, 906, 768
H = 12
D = 64
M1 = 513
N_CORES = 8
BPC = B // N_CORES          # batch items per core
KC = C // 128               # 6 contraction chunks over C
NPAIR = H // 2              # 6 head pairs
NCH = 8                     # key chunks (uniform 113/114)
_kszs = [114, 114] + [113] * 6
_kst = [sum(_kszs[:i]) for i in range(NCH)]
KCH = list(zip(_kst, _kszs))            # key chunks
MIXC = 4                                 # chunk containing key M1-1=512
MIXR = 512 - _kst[MIXC] + 1              # rows [0,MIXR) of chunk 4 are keys <= 512
QCH = KCH                                # query row chunks for out-proj
QP = [(0, 512), (512, N - 512)]          # query column blocks (A, B)
CPASS = [(0, 512), (512, C - 512)]       # column passes over C
SCALE = D ** -0.5

TRACE = False          # set by test.py to capture a HW profile
LAST_RESULTS = None    # BassKernelResults of the most recent run

_BUILT = None


def _install_trace_shim():
    """The image's antenv lacks axon_hooks; recreate it so trace=True works."""
    import sys, types
    if "antenv.axon_hooks" in sys.modules:
        return
    mod = types.ModuleType("antenv.axon_hooks")
    mod._hook = None
    mod.set_axon_ntff_profile_hook = lambda h: setattr(mod, "_hook", h)
    mod.get_axon_ntff_profile_hook = lambda: mod._hook
    sys.modules["antenv.axon_hooks"] = mod
    import antenv
    antenv.axon_hooks = mod
    from trn_agent_boot.trn_boot import _ntff_profile_via_ctypes
    mod.set_axon_ntff_profile_hook(_ntff_profile_via_ctypes("/opt/axon/libaxon_pjrt.so"))


def _build():
    import concourse.tile as tile
    from concourse import bacc, mybir

    BF = mybir.dt.bfloat16
    F32 = mybir.dt.float32
    AF = mybir.ActivationFunctionType

    nc = bacc.Bacc("TRN2", target_bir_lowering=False, debug=False, num_devices=N_CORES)

    xT_d = nc.dram_tensor("xT", [BPC, C, N], BF, kind="ExternalInput").ap()
    w_d = {
        wn: nc.dram_tensor(wn, [C, C], BF, kind="ExternalInput").ap()
        for wn in ("wq", "wk", "wv", "wvc", "wp")
    }
    bias_d = nc.dram_tensor("bias", [128, C], F32, kind="ExternalInput").ap()
    out_d = nc.dram_tensor("out", [BPC, N, C], BF, kind="ExternalOutput").ap()

    with tile.TileContext(nc) as tc:
        from contextlib import ExitStack
        from concourse import library_config

        with ExitStack() as ctx:
            wpool = ctx.enter_context(tc.tile_pool(name="wpool", bufs=1))
            sb = ctx.enter_context(tc.tile_pool(name="sb", bufs=1))
            ps = ctx.enter_context(tc.tile_pool(name="ps", bufs=1, space="PSUM"))

            nc.gpsimd.load_library(library_config.attn)

            mm = nc.tensor.matmul
            pcopy = nc.vector.tensor_copy

            # ---- constants: weights + bias + ones column ----
            w_sb = {}

            def load_w(wn):
                tiles = []
                for kc in range(KC):
                    t = wpool.tile([128, C], BF, name=f"{wn}_{kc}", tag=f"{wn}_{kc}")
                    nc.sync.dma_start(t[:], w_d[wn][kc * 128:(kc + 1) * 128, :])
                    tiles.append(t)
                w_sb[wn] = tiles

            load_w("wq")
            load_w("wk")

            # x^T item 0 first (feeds the first projections); item 1 is DMA'd
            # after the item-0 projections are emitted (slot reuse via tag)
            xT = {}

            def load_xT(it):
                for kc in range(KC):
                    t = sb.tile([128, N], BF, name=f"xT_{it}_{kc}", tag="xT", bufs=8)
                    nc.sync.dma_start(t[:], xT_d[it, kc * 128:(kc + 1) * 128, :])
                    xT[(it, kc)] = t

            load_xT(0)
            load_w("wv")
            load_w("wvc")
            load_w("wp")
            bias_sb = wpool.tile([128, C], F32, name="bias_sb", tag="bias_sb")
            nc.sync.dma_start(bias_sb[:], bias_d[:])
            ones_sb = wpool.tile([128, 1], BF, name="ones_sb", tag="ones_sb")
            nc.vector.memset(ones_sb[:, :], 1.0)
            warm = wpool.tile([128, 1], F32, name="warm", tag="warm")
            nc.scalar.activation(warm[0:1, 0:1], ones_sb[0:1, 0:1], AF.Exp)

            qT, kT, vt, vct, mix, oT = {}, {}, {}, {}, {}, {}

            # ---- projection emitters (each is ~1 psum-bank of PE work) ----
            def qk_block(it, p, wn, blk):
                dst_map = qT if wn == "wq" else kT
                if blk == 0:
                    dst_map[(it, p)] = sb.tile(
                        [128, N], BF, name=f"{wn[1]}T_{it}_{p}",
                        tag="qT" if wn == "wq" else "kT", bufs=7)
                dst = dst_map[(it, p)]
                qs, qw = QP[blk]
                # two half-width sub-groups in the same bank so each PSUM
                # drain copy overlaps the next sub-group's matmuls
                pp = ps.tile([128, 512], F32, name="pp", tag="proj", bufs=1)
                h = (qw + 1) // 2
                for hs, hw in ((0, h), (h, qw - h)):
                    for kc in range(KC):
                        mm(pp[:, hs:hs + hw],
                           lhsT=w_sb[wn][kc][:, p * 128:(p + 1) * 128],
                           rhs=xT[(it, kc)][:, qs + hs:qs + hs + hw],
                           start=(kc == 0), stop=(kc == KC - 1))
                    pcopy(dst[:, qs + hs:qs + hs + hw], pp[:, hs:hs + hw])

            def v_block(it, c, wn, blk):
                dst_map = vt if wn == "wv" else vct
                ts, tsz = KCH[c]
                if blk == 0:
                    dst = sb.tile([128, C], BF, name=f"{wn[1:]}_{it}_{c}",
                                  tag="v" if wn == "wv" else "vc", bufs=16)
                    # AV stationary loads may touch all 128 partitions; keep
                    # the unwritten tail rows finite
                    nc.vector.memset(dst[96:128, :], 0.0)
                    dst_map[(it, c)] = dst
                dst = dst_map[(it, c)]
                cs, cw = CPASS[blk]
                pp = ps.tile([128, 512], F32, name="pp", tag="proj", bufs=1)
                h = cw // 2
                for hs, hw in ((0, h), (h, cw - h)):
                    for kc in range(KC):
                        mm(pp[0:tsz, hs:hs + hw],
                           lhsT=xT[(it, kc)][:, ts:ts + tsz],
                           rhs=w_sb[wn][kc][:, cs + hs:cs + hs + hw],
                           start=(kc == 0), stop=(kc == KC - 1))
                    pcopy(dst[0:tsz, cs + hs:cs + hs + hw], pp[0:tsz, hs:hs + hw])

            def mixes(it):
                # chunk MIXC (keys 454..566) straddles M1: rows [0,MIXR) are
                # keys <= 512 (modality a -> v), the rest modality v -> vc
                am = sb.tile([128, C], BF, name=f"amix_{it}", tag="amix", bufs=2)
                vm = sb.tile([128, C], BF, name=f"vmix_{it}", tag="vmix", bufs=2)
                pcopy(am[:, :], vct[(it, MIXC)][:, :])
                pcopy(am[0:MIXR, :], vt[(it, MIXC)][0:MIXR, :])
                pcopy(vm[:, :], vt[(it, MIXC)][:, :])
                pcopy(vm[0:MIXR, :], vct[(it, MIXC)][0:MIXR, :])
                mix[it] = (am, vm)

            def proj_fillers(it):
                # one list per host attn(0) pair: pair p carries item1's q/k
                # for pair p (whose buffer-slot WAR resolved at pair p-1) plus
                # a share of the v/vc blocks
                qk = [[] for _ in range(NPAIR)]
                for p in range(NPAIR):
                    for wn in ("wq", "wk"):
                        for blk in (0, 1):
                            qk[p].append(lambda it=it, p=p, wn=wn, blk=blk:
                                         qk_block(it, p, wn, blk))
                vs = []
                for c in (MIXC, 0, 1, 2, 3, 5, 6, 7):
                    for wn in ("wv", "wvc"):
                        for blk in (0, 1):
                            vs.append(lambda it=it, c=c, wn=wn, blk=blk:
                                      v_block(it, c, wn, blk))
                    if c == MIXC:
                        vs.append(lambda it=it: mixes(it))
                per_pair = []
                nv = len(vs)
                for p in range(NPAIR):
                    lo = nv * p // NPAIR
                    hi = nv * (p + 1) // NPAIR
                    per_pair.append(qk[p] + vs[lo:hi])
                return per_pair

            def outproj_fillers(it, engs=None):
                fs = []
                for c in range(NCH):
                    for blk in (0, 1):
                        def f(it=it, c=c, blk=blk):
                            ts, tsz = QCH[c]
                            cs, cw = CPASS[blk]
                            pp = ps.tile([128, 512], F32, name="pp", tag="proj", bufs=1)
                            h = cw // 2
                            ob = sb.tile([128, 512], BF, name="ob", tag="ob", bufs=3)
                            for hs, hw in ((0, h), (h, cw - h)):
                                for kp in range(NPAIR):
                                    mm(pp[0:tsz, hs:hs + hw],
                                       lhsT=oT[(it, kp)][:, ts:ts + tsz],
                                       rhs=w_sb["wp"][kp][:, cs + hs:cs + hs + hw],
                                       start=(kp == 0), stop=(kp == NPAIR - 1))
                                nc.vector.tensor_add(
                                    ob[0:tsz, hs:hs + hw], pp[0:tsz, hs:hs + hw],
                                    bias_sb[0:tsz, cs + hs:cs + hs + hw])
                            eng = engs[(2 * c + blk) % len(engs)] if engs else nc.sync
                            eng.dma_start(out_d[it, ts:ts + tsz, cs:cs + cw],
                                          ob[0:tsz, 0:cw])
                        fs.append(f)
                return fs

            # ---- attention for one (item, pair); pops fillers each chunk ----
            def attn_pair(it, p, fillers):
                q_, k_ = qT[(it, p)], kT[(it, p)]
                # accumulators are pre-zeroed and every matmul uses
                # start=False: correct whether PSUM start-zeroing is
                # bank-wide or per-partition (semantics differ between the
                # simulator model and the HW docs)
                av = ps.tile([128, 1024], F32, name="av", tag="av", bufs=1)
                den = ps.tile([128, 512], F32, name="den", tag="den", bufs=1)
                nc.vector.memset(av[:, :], 0.0)
                nc.vector.memset(den[:, :], 0.0)

                def avden(c, ee, eo_):
                    # AV: 4-way col tiling (128x32). av rows 0:64 = head e,
                    # 64:128 = head o; cols 0:512 = queries 0:512 (mod a),
                    # col 512 = query 512 (mod a), cols 513:907 = queries
                    # 512:906 with mod-v values (col 513 is discarded).
                    # PSUM start=1 zero-arms a whole 2KB bank, so only the
                    # FIRST matmul touching each bank starts and only the
                    # LAST stops; interleaved column-tile groups rely on the
                    # per-byte lazy zeroing in between.
                    st, sp = (c == 0), (c == NCH - 1)
                    ksz = KCH[c][1]
                    va = mix[it][0] if c == MIXC else (vt[(it, c)] if c < MIXC else vct[(it, c)])
                    vv = mix[it][1] if c == MIXC else (vct[(it, c)] if c < MIXC else vt[(it, c)])
                    for ho, et in ((0, ee), (1, eo_)):
                        for dh in (0, 1):
                            m0 = ho * 64 + dh * 32
                            wc = p * 128 + m0
                            mm(av[m0:m0 + 32, 0:512], lhsT=va[0:ksz, wc:wc + 32],
                               rhs=et[0:ksz, 0:512], start=False, stop=sp,
                               tile_position=(0, m0), skip_group_check=True)
                            mm(av[m0:m0 + 32, 513:907], lhsT=vv[0:ksz, wc:wc + 32],
                               rhs=et[0:ksz, 512:906], start=False, stop=False,
                               tile_position=(0, m0), skip_group_check=True)
                            mm(av[m0:m0 + 32, 512:513], lhsT=va[0:ksz, wc:wc + 32],
                               rhs=et[0:ksz, 512:513], start=False, stop=sp,
                               tile_position=(0, m0), skip_group_check=True)
                    # denominators: 4 ones-column accumulators (rows 0/32/64/96)
                    mm(den[0:1, 0:512], lhsT=ones_sb[0:ksz, 0:1],
                       rhs=ee[0:ksz, 0:512], start=False, stop=sp,
                       tile_position=(0, 0), skip_group_check=True)
                    mm(den[32:33, 0:394], lhsT=ones_sb[0:ksz, 0:1],
                       rhs=ee[0:ksz, 512:906], start=False, stop=sp,
                       tile_position=(0, 32), skip_group_check=True)
                    mm(den[64:65, 0:512], lhsT=ones_sb[0:ksz, 0:1],
                       rhs=eo_[0:ksz, 0:512], start=False, stop=sp,
                       tile_position=(0, 64), skip_group_check=True)
                    mm(den[96:97, 0:394], lhsT=ones_sb[0:ksz, 0:1],
                       rhs=eo_[0:ksz, 512:906], start=False, stop=sp,
                       tile_position=(0, 96), skip_group_check=True)

                # chunk loop is software-pipelined one deep: AV/den for chunk
                # c-1 are emitted after the scores+exp of chunk c, so the PE
                # never sits behind the exp latency
                prev = None
                skip = 2 if (it == 0 and p == 0) else 0
                for c, (ks, ksz) in enumerate(KCH):
                    # scores: e/o heads row-tiled (64x128), separate bank pairs
                    sc = ps.tile([128, 2048], F32, name="sc", tag="sc", bufs=1)
                    for qs, qw in QP:
                        mm(sc[0:ksz, qs:qs + qw],
                           lhsT=k_[0:64, ks:ks + ksz], rhs=q_[0:64, qs:qs + qw],
                           start=True, stop=True)
                        mm(sc[0:ksz, 1024 + qs:1024 + qs + qw],
                           lhsT=k_[64:128, ks:ks + ksz], rhs=q_[64:128, qs:qs + qw],
                           start=True, stop=True)
                    # per-head exps: the e-head banks free while the o-head
                    # exp still runs, letting the next chunk's e-scores start
                    ee = sb.tile([128, 1024], BF, name="ee", tag="exp", bufs=6)
                    eo_ = sb.tile([128, 1024], BF, name="eo", tag="exp", bufs=6)
                    nc.scalar.activation(ee[0:ksz, 0:906], sc[0:ksz, 0:906],
                                         AF.Exp, scale=SCALE)
                    nc.scalar.activation(eo_[0:ksz, 0:906], sc[0:ksz, 1024:1930],
                                         AF.Exp, scale=SCALE)
                    if prev is not None:
                        avden(*prev)
                    prev = (c, ee, eo_)
                    if fillers and c >= skip:
                        fillers.pop(0)()
                avden(*prev)

                # drain the AV accumulator to SBUF so the next pair's AV can
                # start without waiting on this pair's softmax chain
                avf = sb.tile([128, 1024], F32, name="avf", tag="avf", bufs=2)
                pcopy(avf[:, 0:907], av[:, 0:907])

                # ---- softmax division ----
                rc = sb.tile([128, 512], F32, name="rc", tag="rc", bufs=1)
                nc.vector.reciprocal_approx_fast(rc[0:97, 0:512], den[0:97, 0:512])
                # partition_broadcast reads physical partition 0; relocate the
                # three off-zero reciprocal rows there first
                rl = sb.tile([128, 1300], F32, name="rl", tag="rl", bufs=1)
                nc.sync.dma_start(rl[0:1, 0:394], rc[32:33, 0:394])
                nc.sync.dma_start(rl[0:1, 394:906], rc[64:65, 0:512])
                nc.sync.dma_start(rl[0:1, 906:1300], rc[96:97, 0:394])
                bce = sb.tile([128, N], F32, name="bce", tag="bc", bufs=2)
                bco = sb.tile([128, N], F32, name="bco", tag="bc", bufs=2)
                nc.gpsimd.partition_broadcast(bce[:, 0:512], rc[0:1, 0:512])
                nc.gpsimd.partition_broadcast(bce[:, 512:906], rl[0:1, 0:394])
                nc.gpsimd.partition_broadcast(bco[:, 0:512], rl[0:1, 394:906])
                nc.gpsimd.partition_broadcast(bco[:, 512:906], rl[0:1, 906:1300])
                ot = sb.tile([128, N], BF, name=f"oT_{it}_{p}", tag="oT", bufs=9)
                for rows, bc in ((slice(0, 64), bce), (slice(64, 128), bco)):
                    nc.vector.tensor_mul(ot[rows, 0:513], avf[rows, 0:513],
                                         bc[rows, 0:513])
                    nc.vector.tensor_mul(ot[rows, 513:906], avf[rows, 514:907],
                                         bc[rows, 513:906])
                oT[(it, p)] = ot
                if fillers:
                    fillers.pop(0)()

            # ================= emission =================
            for p in range(NPAIR):
                for blk in (0, 1):
                    qk_block(0, p, "wq", blk)
                for blk in (0, 1):
                    qk_block(0, p, "wk", blk)
            for c in (MIXC, 0, 1, 2, 3, 5, 6, 7):
                for wn in ("wv", "wvc"):
                    for blk in (0, 1):
                        v_block(0, c, wn, blk)
                if c == MIXC:
                    mixes(0)

            load_xT(1)

            fill1 = proj_fillers(1)
            for p in range(NPAIR):
                fl = fill1[p]
                attn_pair(0, p, fl)
                for f in fl:
                    f()

            fill2 = outproj_fillers(0)
            for p in range(NPAIR):
                attn_pair(1, p, fill2)
            for f in fill2:
                f()
            # tail: rotate output DMAs across the idle DGE queues
            for f in outproj_fillers(1, engs=[nc.sync, nc.scalar, nc.gpsimd]):
                f()

    nc.compile()
    return nc


def _get_built():
    global _BUILT
    if _BUILT is None:
        _BUILT = _build()
    return _BUILT


def kernel(x, Wq, Wk, Wv, Wvc, Wp, bp):
    global LAST_RESULTS
    from concourse.bass_utils import run_bass_kernel_spmd

    x = np.asarray(x, dtype=np.float32)
    bf = ml_dtypes.bfloat16
    xT = np.ascontiguousarray(x.transpose(0, 2, 1)).astype(bf)      # (B, C, N)
    ws = {
        "wq": np.asarray(Wq, dtype=np.float32).astype(bf),
        "wk": np.asarray(Wk, dtype=np.float32).astype(bf),
        "wv": np.asarray(Wv, dtype=np.float32).astype(bf),
        "wvc": np.asarray(Wvc, dtype=np.float32).astype(bf),
        "wp": np.asarray(Wp, dtype=np.float32).astype(bf),
    }
    bias = np.ascontiguousarray(
        np.broadcast_to(np.asarray(bp, dtype=np.float32), (128, C))
    )

    if TRACE:
        _install_trace_shim()

    nc = _get_built()
    in_maps = []
    for i in range(N_CORES):
        m = {"xT": np.ascontiguousarray(xT[i * BPC:(i + 1) * BPC]), "bias": bias}
        m.update(ws)
        in_maps.append(m)

    res = run_bass_kernel_spmd(nc, in_maps, list(range(N_CORES)), trace=TRACE,
                               stitch_traces=False)
    LAST_RESULTS = res
    out = np.concatenate([res.results[i]["out"] for i in range(N_CORES)],
                     axis=0).astype(np.float32)
    return out


# revision 12
# speedup vs baseline: 1.4444x; 1.3602x over previous
"""Multi-modality double-value attention on 8 TRN2 NeuronCores.

Sharding: data-parallel over batch (16 items -> 2 per core). Each core runs
the full attention block for its 2 items; weights are replicated. No
collectives. Host pre-transposes x to x^T and casts inputs to bf16; compute
is bf16 with fp32 PSUM accumulation; output is fp32.

v2: PE-array tiling + software pipelining.
 - scores: 2-way row tiling (64x128 mode) - both heads of a pair run
   concurrently on disjoint PE row groups (K=64 each, no zero padding).
 - AV + softmax denominators: 4-way column tiling (128x32 mode) - the two
   heads' value matmuls (2 x M=32 each) and 4 ones-column denominator
   accumulators share the array.
 - exp: one Scalar-engine activation per (pair, key-chunk) covering both
   heads' scores (reads 4 PSUM banks in a single [ksz, 1930] sweep).
 - key chunks are uniform 113/114 so every matmul keeps the same tile size
   (round_up -> 128) - no PE mode changes from ragged tails.
 - item 1's projections are emitted as filler work inside item 0's
   scalar-bound attention loop; item 0's output projection fills item 1's.
"""

import numpy as np
import ml_dtypes

B, N, C = 16, 906, 768
H = 12
D = 64
M1 = 513
N_CORES = 8
BPC = B // N_CORES          # batch items per core
KC = C // 128               # 6 contraction chunks over C
NPAIR = H // 2              # 6 head pairs
NCH = 8                     # key chunks (uniform 113/114)
_kszs = [114, 114] + [113] * 6
_kst = [sum(_kszs[:i]) for i in range(NCH)]
KCH = list(zip(_kst, _kszs))            # key chunks
MIXC = 4                                 # chunk containing key M1-1=512
MIXR = 512 - _kst[MIXC] + 1              # rows [0,MIXR) of chunk 4 are keys <= 512
QCH = KCH                                # query row chunks for out-proj
QP = [(0, 512), (512, N - 512)]          # query column blocks (A, B)
CPASS = [(0, 512), (512, C - 512)]       # column passes over C
SCALE = D ** -0.5

TRACE = False          # set by test.py to capture a HW profile
LAST_RESULTS = None    # BassKernelResults of the most recent run

_BUILT = None


def _install_trace_shim():
    """The image's antenv lacks axon_hooks; recreate it so trace=True works."""
    import sys, types
    if "antenv.axon_hooks" in sys.modules:
        return
    mod = types.ModuleType("antenv.axon_hooks")
    mod._hook = None
    mod.set_axon_ntff_profile_hook = lambda h: setattr(mod, "_hook", h)
    mod.get_axon_ntff_profile_hook = lambda: mod._hook
    sys.modules["antenv.axon_hooks"] = mod
    import antenv
    antenv.axon_hooks = mod
    from trn_agent_boot.trn_boot import _ntff_profile_via_ctypes
    mod.set_axon_ntff_profile_hook(_ntff_profile_via_ctypes("/opt/axon/libaxon_pjrt.so"))


def _build():
    import concourse.tile as tile
    from concourse import bacc, mybir

    BF = mybir.dt.bfloat16
    F32 = mybir.dt.float32
    AF = mybir.ActivationFunctionType

    nc = bacc.Bacc("TRN2", target_bir_lowering=False, debug=False, num_devices=N_CORES)

    xT_d = nc.dram_tensor("xT", [BPC, C, N], BF, kind="ExternalInput").ap()
    w_d = {
        wn: nc.dram_tensor(wn, [C, C], BF, kind="ExternalInput").ap()
        for wn in ("wq", "wk", "wv", "wvc", "wp")
    }
    bias_d = nc.dram_tensor("bias", [128, C], F32, kind="ExternalInput").ap()
    out_d = nc.dram_tensor("out", [BPC, N, C], BF, kind="ExternalOutput").ap()

    with tile.TileContext(nc) as tc:
        from contextlib import ExitStack
        from concourse import library_config

        with ExitStack() as ctx:
            wpool = ctx.enter_context(tc.tile_pool(name="wpool", bufs=1))
            sb = ctx.enter_context(tc.tile_pool(name="sb", bufs=1))
            ps = ctx.enter_context(tc.tile_pool(name="ps", bufs=1, space="PSUM"))

            nc.gpsimd.load_library(library_config.attn)

            mm = nc.tensor.matmul
            pcopy = nc.vector.tensor_copy

            # ---- constants: weights + bias + ones column ----
            w_sb = {}

            def load_w(wn):
                tiles = []
                for kc in range(KC):
                    t = wpool.tile([128, C], BF, name=f"{wn}_{kc}", tag=f"{wn}_{kc}")
                    nc.sync.dma_start(t[:], w_d[wn][kc * 128:(kc + 1) * 128, :])
                    tiles.append(t)
                w_sb[wn] = tiles

            load_w("wq")
            load_w("wk")

            # x^T item 0 first (feeds the first projections); item 1 is DMA'd
            # after the item-0 projections are emitted (slot reuse via tag)
            xT = {}

            def load_xT(it):
                for kc in range(KC):
                    t = sb.tile([128, N], BF, name=f"xT_{it}_{kc}", tag="xT", bufs=8)
                    nc.sync.dma_start(t[:], xT_d[it, kc * 128:(kc + 1) * 128, :])
                    xT[(it, kc)] = t

            load_xT(0)
            load_w("wv")
            load_w("wvc")
            load_w("wp")
            bias_sb = wpool.tile([128, C], F32, name="bias_sb", tag="bias_sb")
            nc.sync.dma_start(bias_sb[:], bias_d[:])
            ones_sb = wpool.tile([128, 1], BF, name="ones_sb", tag="ones_sb")
            nc.vector.memset(ones_sb[:, :], 1.0)
            warm = wpool.tile([128, 1], F32, name="warm", tag="warm")
            nc.scalar.activation(warm[0:1, 0:1], ones_sb[0:1, 0:1], AF.Exp)

            qT, kT, vt, vct, mix, oT = {}, {}, {}, {}, {}, {}

            # ---- segment emitters: each emits ONE 6-matmul psum-bank
            # group + its drain, so consecutive segments in different banks
            # pipeline (a new group in a bank must wait for the previous
            # group's drain-read of that bank)
            QSEG = [(0, 256), (256, 256), (512, 394)]
            VSEG = [(0, 256), (256, 256), (512, 256)]
            OSEG = [(0, 256), (256, 256), (512, 256)]
            obm = {}

            def qk_seg(it, p, wn, qs, qw, pp):
                dst_map = qT if wn == "wq" else kT
                if qs == 0:
                    dst_map[(it, p)] = sb.tile(
                        [128, N], BF, name=f"{wn[1]}T_{it}_{p}",
                        tag="qT" if wn == "wq" else "kT", bufs=7)
                dst = dst_map[(it, p)]
                for kc in range(KC):
                    mm(pp[:, 0:qw],
                       lhsT=w_sb[wn][kc][:, p * 128:(p + 1) * 128],
                       rhs=xT[(it, kc)][:, qs:qs + qw],
                       start=(kc == 0), stop=(kc == KC - 1))
                pcopy(dst[:, qs:qs + qw], pp[:, 0:qw])

            def v_seg(it, c, wn, cs, cw, pp):
                dst_map = vt if wn == "wv" else vct
                ts, tsz = KCH[c]
                if cs == 0:
                    dst = sb.tile([128, C], BF, name=f"{wn[1:]}_{it}_{c}",
                                  tag="v" if wn == "wv" else "vc", bufs=16)
                    # AV stationary loads may touch all 128 partitions; keep
                    # the unwritten tail rows finite
                    nc.vector.memset(dst[96:128, :], 0.0)
                    dst_map[(it, c)] = dst
                dst = dst_map[(it, c)]
                for kc in range(KC):
                    mm(pp[0:tsz, 0:cw],
                       lhsT=xT[(it, kc)][:, ts:ts + tsz],
                       rhs=w_sb[wn][kc][:, cs:cs + cw],
                       start=(kc == 0), stop=(kc == KC - 1))
                pcopy(dst[0:tsz, cs:cs + cw], pp[0:tsz, 0:cw])

            def op_seg(it, c, cs, cw, pp, eng):
                ts, tsz = QCH[c]
                if cs == 0:
                    obm[(it, c)] = sb.tile([128, C], BF, name="ob", tag="ob",
                                           bufs=3)
                ob = obm[(it, c)]
                for kp in range(NPAIR):
                    mm(pp[0:tsz, 0:cw],
                       lhsT=oT[(it, kp)][:, ts:ts + tsz],
                       rhs=w_sb["wp"][kp][:, cs:cs + cw],
                       start=(kp == 0), stop=(kp == NPAIR - 1))
                nc.vector.tensor_add(ob[0:tsz, cs:cs + cw], pp[0:tsz, 0:cw],
                                     bias_sb[0:tsz, cs:cs + cw])
                if cs + cw == C:
                    eng.dma_start(out_d[it, ts:ts + tsz, :], ob[0:tsz, :])

            def mixes(it):
                # chunk MIXC (keys 454..566) straddles M1: rows [0,MIXR) are
                # keys <= 512 (modality a -> v), the rest modality v -> vc
                am = sb.tile([128, C], BF, name=f"amix_{it}", tag="amix", bufs=2)
                vm = sb.tile([128, C], BF, name=f"vmix_{it}", tag="vmix", bufs=2)
                pcopy(am[:, :], vct[(it, MIXC)][:, :])
                pcopy(am[0:MIXR, :], vt[(it, MIXC)][0:MIXR, :])
                pcopy(vm[:, :], vt[(it, MIXC)][:, :])
                pcopy(vm[0:MIXR, :], vct[(it, MIXC)][0:MIXR, :])
                mix[it] = (am, vm)

            def bank_rotator():
                """Rotating 1-bank psum slots carved from the attention tags
                (sc/av/den/proj) - only valid in the standalone projection
                and tail phases where attention psum is idle. 8-deep rotation
                keeps every group's WAR far behind."""
                views = []

                def get():
                    if not views:
                        t = ps.tile([128, 2048], F32, name="scpp", tag="sc",
                                    bufs=1)
                        a = ps.tile([128, 1024], F32, name="avpp", tag="av",
                                    bufs=1)
                        d = ps.tile([128, 512], F32, name="denpp", tag="den",
                                    bufs=1)
                        q = ps.tile([128, 512], F32, name="pp", tag="proj",
                                    bufs=1)
                        views.extend([t[:, 0:512], t[:, 512:1024],
                                      t[:, 1024:1536], t[:, 1536:2048],
                                      a[:, 0:512], a[:, 512:1024], d, q])
                    return views.pop(0)
                return get

            def filler_units(it):
                # per attn(0)-pair unit lists: pair p carries item1's q/k for
                # pair p (buffer-slot WAR resolves at pair p-1) + v/vc share
                def u(fn, *a):
                    return lambda: fn(*a, ps.tile([128, 512], F32, name="pp",
                                                  tag="proj", bufs=1))
                per_pair = [[] for _ in range(NPAIR)]
                for p in range(NPAIR):
                    for wn in ("wq", "wk"):
                        for qs, qw in QSEG:
                            per_pair[p].append(u(qk_seg, it, p, wn, qs, qw))
                vs = []
                for c in (MIXC, 0, 1, 2, 3, 5, 6, 7):
                    for wn in ("wv", "wvc"):
                        for cs, cw in VSEG:
                            vs.append(u(v_seg, it, c, wn, cs, cw))
                    if c == MIXC:
                        vs.append(lambda it=it: mixes(it))
                nv = len(vs)
                for p in range(NPAIR):
                    per_pair[p] += vs[nv * p // NPAIR: nv * (p + 1) // NPAIR]
                return per_pair

            def outproj_units(it, eng):
                def u(fn, *a):
                    return lambda: fn(*a, ps.tile([128, 512], F32, name="pp",
                                                  tag="proj", bufs=1), eng)
                return [u(op_seg, it, c, cs, cw)
                        for c in range(NCH) for cs, cw in OSEG]

            # ---- attention for one (item, pair); pops fillers each chunk ----
            def attn_pair(it, p, fillers, pops=2):
                q_, k_ = qT[(it, p)], kT[(it, p)]
                # accumulators are pre-zeroed and every matmul uses
                # start=False: correct whether PSUM start-zeroing is
                # bank-wide or per-partition (semantics differ between the
                # simulator model and the HW docs)
                av = ps.tile([128, 1024], F32, name="av", tag="av", bufs=1)
                den = ps.tile([128, 512], F32, name="den", tag="den", bufs=1)
                nc.vector.memset(av[:, :], 0.0)
                nc.vector.memset(den[:, :], 0.0)

                def avden(c, ee, eo_):
                    # AV: 4-way col tiling (128x32). av rows 0:64 = head e,
                    # 64:128 = head o; cols 0:512 = queries 0:512 (mod a),
                    # col 512 = query 512 (mod a), cols 513:907 = queries
                    # 512:906 with mod-v values (col 513 is discarded)
                    sp = (c == NCH - 1)
                    ksz = KCH[c][1]
                    va = mix[it][0] if c == MIXC else (vt[(it, c)] if c < MIXC else vct[(it, c)])
                    vv = mix[it][1] if c == MIXC else (vct[(it, c)] if c < MIXC else vt[(it, c)])
                    for ho, et in ((0, ee), (1, eo_)):
                        for dh in (0, 1):
                            m0 = ho * 64 + dh * 32
                            wc = p * 128 + m0
                            mm(av[m0:m0 + 32, 0:512], lhsT=va[0:ksz, wc:wc + 32],
                               rhs=et[0:ksz, 0:512], start=False, stop=sp,
                               tile_position=(0, m0), skip_group_check=True)
                            mm(av[m0:m0 + 32, 513:907], lhsT=vv[0:ksz, wc:wc + 32],
                               rhs=et[0:ksz, 512:906], start=False, stop=False,
                               tile_position=(0, m0), skip_group_check=True)
                            mm(av[m0:m0 + 32, 512:513], lhsT=va[0:ksz, wc:wc + 32],
                               rhs=et[0:ksz, 512:513], start=False, stop=sp,
                               tile_position=(0, m0), skip_group_check=True)
                    # denominators: 4 ones-column accumulators (rows 0/32/64/96)
                    mm(den[0:1, 0:512], lhsT=ones_sb[0:ksz, 0:1],
                       rhs=ee[0:ksz, 0:512], start=False, stop=sp,
                       tile_position=(0, 0), skip_group_check=True)
                    mm(den[32:33, 0:394], lhsT=ones_sb[0:ksz, 0:1],
                       rhs=ee[0:ksz, 512:906], start=False, stop=sp,
                       tile_position=(0, 32), skip_group_check=True)
                    mm(den[64:65, 0:512], lhsT=ones_sb[0:ksz, 0:1],
                       rhs=eo_[0:ksz, 0:512], start=False, stop=sp,
                       tile_position=(0, 64), skip_group_check=True)
                    mm(den[96:97, 0:394], lhsT=ones_sb[0:ksz, 0:1],
                       rhs=eo_[0:ksz, 512:906], start=False, stop=sp,
                       tile_position=(0, 96), skip_group_check=True)

                # chunk loop is software-pipelined one deep: AV/den for chunk
                # c-1 are emitted after the scores+exp of chunk c so the PE
                # never sits behind the exp latency; filler units go between
                # attention groups so their psum WARs hide under real work
                prev = None
                skip = 2 if (it == 0 and p == 0) else 0
                for c, (ks, ksz) in enumerate(KCH):
                    # scores: e/o heads row-tiled (64x128), separate bank pairs
                    sc = ps.tile([128, 2048], F32, name="sc", tag="sc", bufs=1)
                    for qs, qw in QP:
                        mm(sc[0:ksz, qs:qs + qw],
                           lhsT=k_[0:64, ks:ks + ksz], rhs=q_[0:64, qs:qs + qw],
                           start=True, stop=True)
                        mm(sc[0:ksz, 1024 + qs:1024 + qs + qw],
                           lhsT=k_[64:128, ks:ks + ksz], rhs=q_[64:128, qs:qs + qw],
                           start=True, stop=True)
                    # per-head exps: the e-head banks free while the o-head
                    # exp still runs, letting the next chunk's e-scores start
                    ee = sb.tile([128, 1024], BF, name="ee", tag="exp", bufs=6)
                    eo_ = sb.tile([128, 1024], BF, name="eo", tag="exp", bufs=6)
                    nc.scalar.activation(ee[0:ksz, 0:906], sc[0:ksz, 0:906],
                                         AF.Exp, scale=SCALE)
                    nc.scalar.activation(eo_[0:ksz, 0:906], sc[0:ksz, 1024:1930],
                                         AF.Exp, scale=SCALE)
                    if fillers and c >= skip:
                        fillers.pop(0)()
                    if prev is not None:
                        avden(*prev)
                    prev = (c, ee, eo_)
                    if pops > 1 and fillers and c >= skip:
                        fillers.pop(0)()
                avden(*prev)

                # drain the AV accumulator to SBUF so the next pair's AV can
                # start without waiting on this pair's softmax chain
                avf = sb.tile([128, 1024], F32, name="avf", tag="avf", bufs=2)
                pcopy(avf[:, 0:907], av[:, 0:907])

                # ---- softmax division ----
                rc = sb.tile([128, 512], F32, name="rc", tag="rc", bufs=1)
                nc.vector.reciprocal_approx_fast(rc[0:97, 0:512], den[0:97, 0:512])
                # partition_broadcast reads physical partition 0; relocate the
                # three off-zero reciprocal rows there first
                rl = sb.tile([128, 1300], F32, name="rl", tag="rl", bufs=1)
                nc.sync.dma_start(rl[0:1, 0:394], rc[32:33, 0:394])
                nc.sync.dma_start(rl[0:1, 394:906], rc[64:65, 0:512])
                nc.sync.dma_start(rl[0:1, 906:1300], rc[96:97, 0:394])
                bce = sb.tile([128, N], F32, name="bce", tag="bc", bufs=2)
                bco = sb.tile([128, N], F32, name="bco", tag="bc", bufs=2)
                nc.gpsimd.partition_broadcast(bce[:, 0:512], rc[0:1, 0:512])
                nc.gpsimd.partition_broadcast(bce[:, 512:906], rl[0:1, 0:394])
                nc.gpsimd.partition_broadcast(bco[:, 0:512], rl[0:1, 394:906])
                nc.gpsimd.partition_broadcast(bco[:, 512:906], rl[0:1, 906:1300])
                ot = sb.tile([128, N], BF, name=f"oT_{it}_{p}", tag="oT", bufs=9)
                for rows, bc in ((slice(0, 64), bce), (slice(64, 128), bco)):
                    nc.vector.tensor_mul(ot[rows, 0:513], avf[rows, 0:513],
                                         bc[rows, 0:513])
                    nc.vector.tensor_mul(ot[rows, 513:906], avf[rows, 514:907],
                                         bc[rows, 513:906])
                oT[(it, p)] = ot

            # ================= emission =================
            get_pp = bank_rotator()
            for p in range(NPAIR):
                for wn in ("wq", "wk"):
                    for qs, qw in QSEG:
                        qk_seg(0, p, wn, qs, qw, get_pp())
            for c in (MIXC, 0, 1, 2, 3, 5, 6, 7):
                for wn in ("wv", "wvc"):
                    for cs, cw in VSEG:
                        v_seg(0, c, wn, cs, cw, get_pp())
                if c == MIXC:
                    mixes(0)

            load_xT(1)

            fill1 = filler_units(1)
            for p in range(NPAIR):
                fl = fill1[p]
                attn_pair(0, p, fl, pops=2)
                for f in fl:
                    f()

            fill2 = outproj_units(0, nc.sync)
            for p in range(NPAIR):
                attn_pair(1, p, fill2, pops=1)
            for f in fill2:
                f()
            # tail: item 1's output projection on rotating psum banks and
            # rotating DGE queues (everything else is idle by now)
            get_pp = bank_rotator()
            engs = [nc.sync, nc.scalar, nc.gpsimd]
            for c in range(NCH):
                for cs, cw in OSEG:
                    op_seg(1, c, cs, cw, get_pp(), engs[c % 3])

    nc.compile()
    return nc


def _get_built():
    global _BUILT
    if _BUILT is None:
        _BUILT = _build()
    return _BUILT


def kernel(x, Wq, Wk, Wv, Wvc, Wp, bp):
    global LAST_RESULTS
    from concourse.bass_utils import run_bass_kernel_spmd

    x = np.asarray(x, dtype=np.float32)
    bf = ml_dtypes.bfloat16
    xT = np.ascontiguousarray(x.transpose(0, 2, 1)).astype(bf)      # (B, C, N)
    ws = {
        "wq": np.asarray(Wq, dtype=np.float32).astype(bf),
        "wk": np.asarray(Wk, dtype=np.float32).astype(bf),
        "wv": np.asarray(Wv, dtype=np.float32).astype(bf),
        "wvc": np.asarray(Wvc, dtype=np.float32).astype(bf),
        "wp": np.asarray(Wp, dtype=np.float32).astype(bf),
    }
    bias = np.ascontiguousarray(
        np.broadcast_to(np.asarray(bp, dtype=np.float32), (128, C))
    )

    if TRACE:
        _install_trace_shim()

    nc = _get_built()
    in_maps = []
    for i in range(N_CORES):
        m = {"xT": np.ascontiguousarray(xT[i * BPC:(i + 1) * BPC]), "bias": bias}
        m.update(ws)
        in_maps.append(m)

    res = run_bass_kernel_spmd(nc, in_maps, list(range(N_CORES)), trace=TRACE,
                               stitch_traces=False)
    LAST_RESULTS = res
    out = np.concatenate([res.results[i]["out"] for i in range(N_CORES)],
                     axis=0).astype(np.float32)
    return out


# revision 13
# speedup vs baseline: 1.5754x; 1.0907x over previous
"""Multi-modality double-value attention on 8 TRN2 NeuronCores.

Sharding: data-parallel over batch (16 items -> 2 per core). Each core runs
the full attention block for its 2 items; weights are replicated. No
collectives. Host pre-transposes x to x^T and casts inputs to bf16; compute
is bf16 with fp32 PSUM accumulation; output is fp32.

v2: PE-array tiling + software pipelining.
 - scores: 2-way row tiling (64x128 mode) - both heads of a pair run
   concurrently on disjoint PE row groups (K=64 each, no zero padding).
 - AV + softmax denominators: 4-way column tiling (128x32 mode) - the two
   heads' value matmuls (2 x M=32 each) and 4 ones-column denominator
   accumulators share the array.
 - exp: one Scalar-engine activation per (pair, key-chunk) covering both
   heads' scores (reads 4 PSUM banks in a single [ksz, 1930] sweep).
 - key chunks are uniform 113/114 so every matmul keeps the same tile size
   (round_up -> 128) - no PE mode changes from ragged tails.
 - item 1's projections are emitted as filler work inside item 0's
   scalar-bound attention loop; item 0's output projection fills item 1's.
"""

import numpy as np
import ml_dtypes

B, N, C = 16, 906, 768
H = 12
D = 64
M1 = 513
N_CORES = 8
BPC = B // N_CORES          # batch items per core
KC = C // 128               # 6 contraction chunks over C
NPAIR = H // 2              # 6 head pairs
NCH = 8                     # key chunks (uniform 113/114)
_kszs = [114, 114] + [113] * 6
_kst = [sum(_kszs[:i]) for i in range(NCH)]
KCH = list(zip(_kst, _kszs))            # key chunks
MIXC = 4                                 # chunk containing key M1-1=512
MIXR = 512 - _kst[MIXC] + 1              # rows [0,MIXR) of chunk 4 are keys <= 512
QCH = [(i * 128, min(128, N - i * 128)) for i in range(NCH)]  # out-proj row chunks (128 rows -> out DMAs fan across all 16 queues)
QP = [(0, 512), (512, N - 512)]          # query column blocks (A, B)
CPASS = [(0, 512), (512, C - 512)]       # column passes over C
SCALE = D ** -0.5

TRACE = False          # set by test.py to capture a HW profile
LAST_RESULTS = None    # BassKernelResults of the most recent run

_BUILT = None


def _install_trace_shim():
    """The image's antenv lacks axon_hooks; recreate it so trace=True works."""
    import sys, types
    if "antenv.axon_hooks" in sys.modules:
        return
    mod = types.ModuleType("antenv.axon_hooks")
    mod._hook = None
    mod.set_axon_ntff_profile_hook = lambda h: setattr(mod, "_hook", h)
    mod.get_axon_ntff_profile_hook = lambda: mod._hook
    sys.modules["antenv.axon_hooks"] = mod
    import antenv
    antenv.axon_hooks = mod
    from trn_agent_boot.trn_boot import _ntff_profile_via_ctypes
    mod.set_axon_ntff_profile_hook(_ntff_profile_via_ctypes("/opt/axon/libaxon_pjrt.so"))


def _build():
    import concourse.tile as tile
    from concourse import bacc, mybir

    BF = mybir.dt.bfloat16
    F32 = mybir.dt.float32
    AF = mybir.ActivationFunctionType

    nc = bacc.Bacc("TRN2", target_bir_lowering=False, debug=False, num_devices=N_CORES)

    xT_d = nc.dram_tensor("xT", [BPC, C, N], BF, kind="ExternalInput").ap()
    w_d = {
        wn: nc.dram_tensor(wn, [C, C], BF, kind="ExternalInput").ap()
        for wn in ("wq", "wk", "wv", "wvc", "wp")
    }
    bias_d = nc.dram_tensor("bias", [128, C], F32, kind="ExternalInput").ap()
    out_d = nc.dram_tensor("out", [BPC, N, C], BF, kind="ExternalOutput").ap()

    with tile.TileContext(nc) as tc:
        from contextlib import ExitStack
        from concourse import library_config

        with ExitStack() as ctx:
            wpool = ctx.enter_context(tc.tile_pool(name="wpool", bufs=1))
            sb = ctx.enter_context(tc.tile_pool(name="sb", bufs=1))
            ps = ctx.enter_context(tc.tile_pool(name="ps", bufs=1, space="PSUM"))

            nc.gpsimd.load_library(library_config.attn)

            mm = nc.tensor.matmul
            pcopy = nc.vector.tensor_copy

            # ---- constants: weights + bias + ones column ----
            w_sb = {}

            def load_w(wn):
                tiles = []
                for kc in range(KC):
                    t = wpool.tile([128, C], BF, name=f"{wn}_{kc}", tag=f"{wn}_{kc}")
                    nc.sync.dma_start(t[:], w_d[wn][kc * 128:(kc + 1) * 128, :])
                    tiles.append(t)
                w_sb[wn] = tiles

            load_w("wq")
            load_w("wk")

            # x^T item 0 first (feeds the first projections); item 1 is DMA'd
            # after the item-0 projections are emitted (slot reuse via tag)
            xT = {}

            def load_xT(it):
                for kc in range(KC):
                    t = sb.tile([128, N], BF, name=f"xT_{it}_{kc}", tag="xT", bufs=8)
                    nc.sync.dma_start(t[:], xT_d[it, kc * 128:(kc + 1) * 128, :])
                    xT[(it, kc)] = t

            load_xT(0)
            load_w("wv")
            load_w("wvc")
            load_w("wp")
            bias_sb = wpool.tile([128, C], F32, name="bias_sb", tag="bias_sb")
            nc.sync.dma_start(bias_sb[:], bias_d[:])
            ones_sb = wpool.tile([128, 1], BF, name="ones_sb", tag="ones_sb")
            nc.vector.memset(ones_sb[:, :], 1.0)
            warm = wpool.tile([128, 1], F32, name="warm", tag="warm")
            nc.scalar.activation(warm[0:1, 0:1], ones_sb[0:1, 0:1], AF.Exp)

            qT, kT, vt, vct, mix, oT = {}, {}, {}, {}, {}, {}

            # ---- segment emitters: each emits ONE 6-matmul psum-bank
            # group + its drain, so consecutive segments in different banks
            # pipeline (a new group in a bank must wait for the previous
            # group's drain-read of that bank)
            QSEG = [(0, 256), (256, 256), (512, 394)]
            VSEG = [(0, 256), (256, 256), (512, 256)]
            OSEG = [(0, 256), (256, 256), (512, 256)]
            obm = {}

            def qk_seg(it, p, wn, qs, qw, pp):
                dst_map = qT if wn == "wq" else kT
                if qs == 0:
                    dst_map[(it, p)] = sb.tile(
                        [128, N], BF, name=f"{wn[1]}T_{it}_{p}",
                        tag="qT" if wn == "wq" else "kT", bufs=7)
                dst = dst_map[(it, p)]
                for kc in range(KC):
                    mm(pp[:, 0:qw],
                       lhsT=w_sb[wn][kc][:, p * 128:(p + 1) * 128],
                       rhs=xT[(it, kc)][:, qs:qs + qw],
                       start=(kc == 0), stop=(kc == KC - 1))
                pcopy(dst[:, qs:qs + qw], pp[:, 0:qw])

            def v_seg(it, c, wn, cs, cw, pp):
                dst_map = vt if wn == "wv" else vct
                ts, tsz = KCH[c]
                if cs == 0:
                    dst = sb.tile([128, C], BF, name=f"{wn[1:]}_{it}_{c}",
                                  tag="v" if wn == "wv" else "vc", bufs=16)
                    # AV stationary loads may touch all 128 partitions; keep
                    # the unwritten tail rows finite
                    nc.vector.memset(dst[96:128, :], 0.0)
                    dst_map[(it, c)] = dst
                dst = dst_map[(it, c)]
                for kc in range(KC):
                    mm(pp[0:tsz, 0:cw],
                       lhsT=xT[(it, kc)][:, ts:ts + tsz],
                       rhs=w_sb[wn][kc][:, cs:cs + cw],
                       start=(kc == 0), stop=(kc == KC - 1))
                pcopy(dst[0:tsz, cs:cs + cw], pp[0:tsz, 0:cw])

            def op_seg(it, c, cs, cw, pp, eng):
                ts, tsz = QCH[c]
                if cs == 0:
                    obm[(it, c)] = sb.tile([128, C], BF, name="ob", tag="ob",
                                           bufs=3)
                ob = obm[(it, c)]
                for kp in range(NPAIR):
                    mm(pp[0:tsz, 0:cw],
                       lhsT=oT[(it, kp)][:, ts:ts + tsz],
                       rhs=w_sb["wp"][kp][:, cs:cs + cw],
                       start=(kp == 0), stop=(kp == NPAIR - 1))
                nc.vector.tensor_add(ob[0:tsz, cs:cs + cw], pp[0:tsz, 0:cw],
                                     bias_sb[0:tsz, cs:cs + cw])
                if cs + cw == C:
                    eng.dma_start(out_d[it, ts:ts + tsz, :], ob[0:tsz, :])

            def mixes(it):
                # chunk MIXC (keys 454..566) straddles M1: rows [0,MIXR) are
                # keys <= 512 (modality a -> v), the rest modality v -> vc
                am = sb.tile([128, C], BF, name=f"amix_{it}", tag="amix", bufs=2)
                vm = sb.tile([128, C], BF, name=f"vmix_{it}", tag="vmix", bufs=2)
                pcopy(am[:, :], vct[(it, MIXC)][:, :])
                pcopy(am[0:MIXR, :], vt[(it, MIXC)][0:MIXR, :])
                pcopy(vm[:, :], vt[(it, MIXC)][:, :])
                pcopy(vm[0:MIXR, :], vct[(it, MIXC)][0:MIXR, :])
                mix[it] = (am, vm)

            def bank_rotator():
                """Rotating 1-bank psum slots carved from the attention tags
                (sc/av/den/proj) - only valid in the standalone projection
                and tail phases where attention psum is idle. 8-deep rotation
                keeps every group's WAR far behind."""
                views = []

                def get():
                    if not views:
                        t = ps.tile([128, 2048], F32, name="scpp", tag="sc",
                                    bufs=1)
                        a = ps.tile([128, 1024], F32, name="avpp", tag="av",
                                    bufs=1)
                        d = ps.tile([128, 512], F32, name="denpp", tag="den",
                                    bufs=1)
                        q = ps.tile([128, 512], F32, name="pp", tag="proj",
                                    bufs=1)
                        views.extend([t[:, 0:512], t[:, 512:1024],
                                      t[:, 1024:1536], t[:, 1536:2048],
                                      a[:, 0:512], a[:, 512:1024], d, q])
                    return views.pop(0)
                return get

            def filler_units(it):
                # per attn(0)-pair unit lists: pair p carries item1's q/k for
                # pair p (buffer-slot WAR resolves at pair p-1) + v/vc share
                def u(fn, *a):
                    return lambda: fn(*a, ps.tile([128, 512], F32, name="pp",
                                                  tag="proj", bufs=1))
                per_pair = [[] for _ in range(NPAIR)]
                for p in range(NPAIR):
                    for wn in ("wq", "wk"):
                        for qs, qw in QSEG:
                            per_pair[p].append(u(qk_seg, it, p, wn, qs, qw))
                vs = []
                for c in (MIXC, 0, 1, 2, 3, 5, 6, 7):
                    for wn in ("wv", "wvc"):
                        for cs, cw in VSEG:
                            vs.append(u(v_seg, it, c, wn, cs, cw))
                    if c == MIXC:
                        vs.append(lambda it=it: mixes(it))
                nv = len(vs)
                for p in range(NPAIR):
                    per_pair[p] += vs[nv * p // NPAIR: nv * (p + 1) // NPAIR]
                return per_pair

            def outproj_units(it, eng):
                def u(fn, *a):
                    return lambda: fn(*a, ps.tile([128, 512], F32, name="pp",
                                                  tag="proj", bufs=1), eng)
                return [u(op_seg, it, c, cs, cw)
                        for c in range(NCH) for cs, cw in OSEG]

            # ---- attention for one (item, pair); pops fillers each chunk ----
            def attn_pair(it, p, fillers, pops=2):
                q_, k_ = qT[(it, p)], kT[(it, p)]
                # accumulators are pre-zeroed and every matmul uses
                # start=False: correct whether PSUM start-zeroing is
                # bank-wide or per-partition (semantics differ between the
                # simulator model and the HW docs)
                av = ps.tile([128, 1024], F32, name="av", tag="av", bufs=1)
                den = ps.tile([128, 512], F32, name="den", tag="den", bufs=1)
                nc.vector.memset(av[:, :], 0.0)
                nc.vector.memset(den[:, :], 0.0)

                def avden(c, ee, eo_):
                    # AV: 4-way col tiling (128x32). av rows 0:64 = head e,
                    # 64:128 = head o; cols 0:512 = queries 0:512 (mod a),
                    # col 512 = query 512 (mod a), cols 513:907 = queries
                    # 512:906 with mod-v values (col 513 is discarded)
                    sp = (c == NCH - 1)
                    ksz = KCH[c][1]
                    va = mix[it][0] if c == MIXC else (vt[(it, c)] if c < MIXC else vct[(it, c)])
                    vv = mix[it][1] if c == MIXC else (vct[(it, c)] if c < MIXC else vt[(it, c)])
                    for ho, et in ((0, ee), (1, eo_)):
                        for dh in (0, 1):
                            m0 = ho * 64 + dh * 32
                            wc = p * 128 + m0
                            mm(av[m0:m0 + 32, 0:512], lhsT=va[0:ksz, wc:wc + 32],
                               rhs=et[0:ksz, 0:512], start=False, stop=sp,
                               tile_position=(0, m0), skip_group_check=True)
                            mm(av[m0:m0 + 32, 513:907], lhsT=vv[0:ksz, wc:wc + 32],
                               rhs=et[0:ksz, 512:906], start=False, stop=False,
                               tile_position=(0, m0), skip_group_check=True)
                            mm(av[m0:m0 + 32, 512:513], lhsT=va[0:ksz, wc:wc + 32],
                               rhs=et[0:ksz, 512:513], start=False, stop=sp,
                               tile_position=(0, m0), skip_group_check=True)
                    # denominators: 4 ones-column accumulators (rows 0/32/64/96)
                    mm(den[0:1, 0:512], lhsT=ones_sb[0:ksz, 0:1],
                       rhs=ee[0:ksz, 0:512], start=False, stop=sp,
                       tile_position=(0, 0), skip_group_check=True)
                    mm(den[32:33, 0:394], lhsT=ones_sb[0:ksz, 0:1],
                       rhs=ee[0:ksz, 512:906], start=False, stop=sp,
                       tile_position=(0, 32), skip_group_check=True)
                    mm(den[64:65, 0:512], lhsT=ones_sb[0:ksz, 0:1],
                       rhs=eo_[0:ksz, 0:512], start=False, stop=sp,
                       tile_position=(0, 64), skip_group_check=True)
                    mm(den[96:97, 0:394], lhsT=ones_sb[0:ksz, 0:1],
                       rhs=eo_[0:ksz, 512:906], start=False, stop=sp,
                       tile_position=(0, 96), skip_group_check=True)

                # chunk loop is software-pipelined one deep: AV/den for chunk
                # c-1 are emitted after the scores+exp of chunk c so the PE
                # never sits behind the exp latency; filler units go between
                # attention groups so their psum WARs hide under real work
                prev = None
                skip = 2 if (it == 0 and p == 0) else 0
                for c, (ks, ksz) in enumerate(KCH):
                    # scores: e/o heads row-tiled (64x128), separate bank pairs
                    sc = ps.tile([128, 2048], F32, name="sc", tag="sc", bufs=1)
                    for qs, qw in QP:
                        mm(sc[0:ksz, qs:qs + qw],
                           lhsT=k_[0:64, ks:ks + ksz], rhs=q_[0:64, qs:qs + qw],
                           start=True, stop=True)
                        mm(sc[0:ksz, 1024 + qs:1024 + qs + qw],
                           lhsT=k_[64:128, ks:ks + ksz], rhs=q_[64:128, qs:qs + qw],
                           start=True, stop=True)
                    # per-head exps: the e-head banks free while the o-head
                    # exp still runs, letting the next chunk's e-scores start
                    ee = sb.tile([128, 1024], BF, name="ee", tag="exp", bufs=6)
                    eo_ = sb.tile([128, 1024], BF, name="eo", tag="exp", bufs=6)
                    nc.scalar.activation(ee[0:ksz, 0:906], sc[0:ksz, 0:906],
                                         AF.Exp, scale=SCALE)
                    nc.scalar.activation(eo_[0:ksz, 0:906], sc[0:ksz, 1024:1930],
                                         AF.Exp, scale=SCALE)
                    if fillers and c >= skip:
                        fillers.pop(0)()
                    if prev is not None:
                        avden(*prev)
                    prev = (c, ee, eo_)
                    if pops > 1 and fillers and c >= skip:
                        fillers.pop(0)()
                avden(*prev)

                # drain the AV accumulator to SBUF so the next pair's AV can
                # start without waiting on this pair's softmax chain
                avf = sb.tile([128, 1024], F32, name="avf", tag="avf", bufs=2)
                pcopy(avf[:, 0:907], av[:, 0:907])

                # ---- softmax division ----
                rc = sb.tile([128, 512], F32, name="rc", tag="rc", bufs=1)
                nc.vector.reciprocal_approx_fast(rc[0:97, 0:512], den[0:97, 0:512])
                # partition_broadcast reads physical partition 0; relocate the
                # three off-zero reciprocal rows there first
                rl = sb.tile([128, 1300], F32, name="rl", tag="rl", bufs=1)
                nc.sync.dma_start(rl[0:1, 0:394], rc[32:33, 0:394])
                nc.sync.dma_start(rl[0:1, 394:906], rc[64:65, 0:512])
                nc.sync.dma_start(rl[0:1, 906:1300], rc[96:97, 0:394])
                bce = sb.tile([128, N], F32, name="bce", tag="bc", bufs=2)
                bco = sb.tile([128, N], F32, name="bco", tag="bc", bufs=2)
                nc.gpsimd.partition_broadcast(bce[:, 0:512], rc[0:1, 0:512])
                nc.gpsimd.partition_broadcast(bce[:, 512:906], rl[0:1, 0:394])
                nc.gpsimd.partition_broadcast(bco[:, 0:512], rl[0:1, 394:906])
                nc.gpsimd.partition_broadcast(bco[:, 512:906], rl[0:1, 906:1300])
                ot = sb.tile([128, N], BF, name=f"oT_{it}_{p}", tag="oT", bufs=9)
                for rows, bc in ((slice(0, 64), bce), (slice(64, 128), bco)):
                    nc.vector.tensor_mul(ot[rows, 0:513], avf[rows, 0:513],
                                         bc[rows, 0:513])
                    nc.vector.tensor_mul(ot[rows, 513:906], avf[rows, 514:907],
                                         bc[rows, 513:906])
                oT[(it, p)] = ot

            # ================= emission =================
            get_pp = bank_rotator()
            for p in range(NPAIR):
                for wn in ("wq", "wk"):
                    for qs, qw in QSEG:
                        qk_seg(0, p, wn, qs, qw, get_pp())
            for c in (MIXC, 0, 1, 2, 3, 5, 6, 7):
                for wn in ("wv", "wvc"):
                    for cs, cw in VSEG:
                        v_seg(0, c, wn, cs, cw, get_pp())
                if c == MIXC:
                    mixes(0)

            load_xT(1)

            fill1 = filler_units(1)
            for p in range(NPAIR):
                fl = fill1[p]
                attn_pair(0, p, fl, pops=2)
                for f in fl:
                    f()

            fill2 = outproj_units(0, nc.sync)
            for p in range(NPAIR):
                attn_pair(1, p, fill2, pops=1)
            for f in fill2:
                f()
            # tail: item 1's output projection on rotating psum banks and
            # rotating DGE queues (everything else is idle by now)
            get_pp = bank_rotator()
            engs = [nc.sync, nc.scalar, nc.gpsimd]
            for c in range(NCH):
                for cs, cw in OSEG:
                    op_seg(1, c, cs, cw, get_pp(), engs[c % 3])

    nc.compile()
    return nc


def _get_built():
    global _BUILT
    if _BUILT is None:
        _BUILT = _build()
    return _BUILT


def kernel(x, Wq, Wk, Wv, Wvc, Wp, bp):
    global LAST_RESULTS
    from concourse.bass_utils import run_bass_kernel_spmd

    x = np.asarray(x, dtype=np.float32)
    bf = ml_dtypes.bfloat16
    xT = np.ascontiguousarray(x.transpose(0, 2, 1)).astype(bf)      # (B, C, N)
    ws = {
        "wq": np.asarray(Wq, dtype=np.float32).astype(bf),
        "wk": np.asarray(Wk, dtype=np.float32).astype(bf),
        "wv": np.asarray(Wv, dtype=np.float32).astype(bf),
        "wvc": np.asarray(Wvc, dtype=np.float32).astype(bf),
        "wp": np.asarray(Wp, dtype=np.float32).astype(bf),
    }
    bias = np.ascontiguousarray(
        np.broadcast_to(np.asarray(bp, dtype=np.float32), (128, C))
    )

    if TRACE:
        _install_trace_shim()

    nc = _get_built()
    in_maps = []
    for i in range(N_CORES):
        m = {"xT": np.ascontiguousarray(xT[i * BPC:(i + 1) * BPC]), "bias": bias}
        m.update(ws)
        in_maps.append(m)

    res = run_bass_kernel_spmd(nc, in_maps, list(range(N_CORES)), trace=TRACE,
                               stitch_traces=False)
    LAST_RESULTS = res
    out = np.concatenate([res.results[i]["out"] for i in range(N_CORES)],
                     axis=0).astype(np.float32)
    return out


# revision 15
# speedup vs baseline: 1.6298x; 1.0345x over previous
"""Multi-modality double-value attention on 8 TRN2 NeuronCores.

Sharding: data-parallel over batch (16 items -> 2 per core). Each core runs
the full attention block for its 2 items; weights are replicated. No
collectives. Host pre-transposes x to x^T and casts inputs to bf16; compute
is bf16 with fp32 PSUM accumulation; output is fp32.

v2: PE-array tiling + software pipelining.
 - scores: 2-way row tiling (64x128 mode) - both heads of a pair run
   concurrently on disjoint PE row groups (K=64 each, no zero padding).
 - AV + softmax denominators: 4-way column tiling (128x32 mode) - the two
   heads' value matmuls (2 x M=32 each) and 4 ones-column denominator
   accumulators share the array.
 - exp: one Scalar-engine activation per (pair, key-chunk) covering both
   heads' scores (reads 4 PSUM banks in a single [ksz, 1930] sweep).
 - key chunks are uniform 113/114 so every matmul keeps the same tile size
   (round_up -> 128) - no PE mode changes from ragged tails.
 - item 1's projections are emitted as filler work inside item 0's
   scalar-bound attention loop; item 0's output projection fills item 1's.
"""

import numpy as np
import ml_dtypes

B, N, C = 16, 906, 768
H = 12
D = 64
M1 = 513
N_CORES = 8
BPC = B // N_CORES          # batch items per core
KC = C // 128               # 6 contraction chunks over C
NPAIR = H // 2              # 6 head pairs
NCH = 8                     # key chunks (uniform 113/114)
_kszs = [114, 114] + [113] * 6
_kst = [sum(_kszs[:i]) for i in range(NCH)]
KCH = list(zip(_kst, _kszs))            # key chunks
MIXC = 4                                 # chunk containing key M1-1=512
MIXR = 512 - _kst[MIXC] + 1              # rows [0,MIXR) of chunk 4 are keys <= 512
QCH = [(i * 128, min(128, N - i * 128)) for i in range(NCH)]  # out-proj row chunks (128 rows -> out DMAs fan across all 16 queues)
QP = [(0, 512), (512, N - 512)]          # query column blocks (A, B)
CPASS = [(0, 512), (512, C - 512)]       # column passes over C
SCALE = D ** -0.5

TRACE = False          # set by test.py to capture a HW profile
LAST_RESULTS = None    # BassKernelResults of the most recent run

_BUILT = None


def _install_trace_shim():
    """The image's antenv lacks axon_hooks; recreate it so trace=True works."""
    import sys, types
    if "antenv.axon_hooks" in sys.modules:
        return
    mod = types.ModuleType("antenv.axon_hooks")
    mod._hook = None
    mod.set_axon_ntff_profile_hook = lambda h: setattr(mod, "_hook", h)
    mod.get_axon_ntff_profile_hook = lambda: mod._hook
    sys.modules["antenv.axon_hooks"] = mod
    import antenv
    antenv.axon_hooks = mod
    from trn_agent_boot.trn_boot import _ntff_profile_via_ctypes
    mod.set_axon_ntff_profile_hook(_ntff_profile_via_ctypes("/opt/axon/libaxon_pjrt.so"))


def _build():
    import concourse.tile as tile
    from concourse import bacc, mybir

    BF = mybir.dt.bfloat16
    F32 = mybir.dt.float32
    AF = mybir.ActivationFunctionType

    nc = bacc.Bacc("TRN2", target_bir_lowering=False, debug=False, num_devices=N_CORES)

    xT_d = nc.dram_tensor("xT", [BPC, C, N], BF, kind="ExternalInput").ap()
    w_d = {
        wn: nc.dram_tensor(wn, [C, C], BF, kind="ExternalInput").ap()
        for wn in ("wq", "wk", "wv", "wvc", "wp")
    }
    bias_d = nc.dram_tensor("bias", [128, C], F32, kind="ExternalInput").ap()
    out_d = nc.dram_tensor("out", [BPC, N, C], BF, kind="ExternalOutput").ap()

    with tile.TileContext(nc) as tc:
        from contextlib import ExitStack
        from concourse import library_config

        with ExitStack() as ctx:
            wpool = ctx.enter_context(tc.tile_pool(name="wpool", bufs=1))
            sb = ctx.enter_context(tc.tile_pool(name="sb", bufs=1))
            ps = ctx.enter_context(tc.tile_pool(name="ps", bufs=1, space="PSUM"))

            nc.gpsimd.load_library(library_config.attn)

            mm = nc.tensor.matmul
            pcopy = nc.vector.tensor_copy

            # ---- constants: weights + bias + ones column ----
            w_sb = {}

            def load_w(wn):
                tiles = []
                for kc in range(KC):
                    t = wpool.tile([128, C], BF, name=f"{wn}_{kc}", tag=f"{wn}_{kc}")
                    nc.sync.dma_start(t[:], w_d[wn][kc * 128:(kc + 1) * 128, :])
                    tiles.append(t)
                w_sb[wn] = tiles

            load_w("wq")

            # x^T item 0 right after wq (they feed the first projections);
            # item 1 is DMA'd after the item-0 projections are emitted
            xT = {}

            def load_xT(it):
                for kc in range(KC):
                    t = sb.tile([128, N], BF, name=f"xT_{it}_{kc}", tag="xT", bufs=8)
                    nc.sync.dma_start(t[:], xT_d[it, kc * 128:(kc + 1) * 128, :])
                    xT[(it, kc)] = t

            load_xT(0)
            load_w("wk")
            load_w("wv")
            load_w("wvc")
            load_w("wp")
            bias_sb = wpool.tile([128, C], F32, name="bias_sb", tag="bias_sb")
            nc.sync.dma_start(bias_sb[:], bias_d[:])
            ones_sb = wpool.tile([128, 1], BF, name="ones_sb", tag="ones_sb")
            nc.vector.memset(ones_sb[:, :], 1.0)
            warm = wpool.tile([128, 1], F32, name="warm", tag="warm")
            nc.scalar.activation(warm[0:1, 0:1], ones_sb[0:1, 0:1], AF.Exp)

            qT, kT, vt, vct, mix, oT = {}, {}, {}, {}, {}, {}

            # ---- segment emitters: each emits ONE 6-matmul psum-bank
            # group + its drain, so consecutive segments in different banks
            # pipeline (a new group in a bank must wait for the previous
            # group's drain-read of that bank)
            QSEG = [(0, 256), (256, 256), (512, 394)]
            VSEG = [(0, 256), (256, 256), (512, 256)]
            OSEG = [(0, 256), (256, 256), (512, 256)]
            obm = {}

            def qk_seg(it, p, wn, qs, qw, pp):
                dst_map = qT if wn == "wq" else kT
                if qs == 0:
                    dst_map[(it, p)] = sb.tile(
                        [128, N], BF, name=f"{wn[1]}T_{it}_{p}",
                        tag="qT" if wn == "wq" else "kT", bufs=7)
                dst = dst_map[(it, p)]
                for kc in range(KC):
                    mm(pp[:, 0:qw],
                       lhsT=w_sb[wn][kc][:, p * 128:(p + 1) * 128],
                       rhs=xT[(it, kc)][:, qs:qs + qw],
                       start=(kc == 0), stop=(kc == KC - 1))
                pcopy(dst[:, qs:qs + qw], pp[:, 0:qw])

            def v_seg(it, c, wn, cs, cw, pp):
                dst_map = vt if wn == "wv" else vct
                ts, tsz = KCH[c]
                if cs == 0:
                    dst = sb.tile([128, C], BF, name=f"{wn[1:]}_{it}_{c}",
                                  tag="v" if wn == "wv" else "vc", bufs=16)
                    # AV stationary loads may touch all 128 partitions; keep
                    # the unwritten tail rows finite
                    nc.vector.memset(dst[96:128, :], 0.0)
                    dst_map[(it, c)] = dst
                dst = dst_map[(it, c)]
                for kc in range(KC):
                    mm(pp[0:tsz, 0:cw],
                       lhsT=xT[(it, kc)][:, ts:ts + tsz],
                       rhs=w_sb[wn][kc][:, cs:cs + cw],
                       start=(kc == 0), stop=(kc == KC - 1))
                pcopy(dst[0:tsz, cs:cs + cw], pp[0:tsz, 0:cw])

            def op_seg(it, c, cs, cw, pp, eng):
                ts, tsz = QCH[c]
                if cs == 0:
                    obm[(it, c)] = sb.tile([128, C], BF, name="ob", tag="ob",
                                           bufs=3)
                ob = obm[(it, c)]
                for kp in range(NPAIR):
                    mm(pp[0:tsz, 0:cw],
                       lhsT=oT[(it, kp)][:, ts:ts + tsz],
                       rhs=w_sb["wp"][kp][:, cs:cs + cw],
                       start=(kp == 0), stop=(kp == NPAIR - 1))
                nc.vector.tensor_add(ob[0:tsz, cs:cs + cw], pp[0:tsz, 0:cw],
                                     bias_sb[0:tsz, cs:cs + cw])
                if cs + cw == C:
                    eng.dma_start(out_d[it, ts:ts + tsz, :], ob[0:tsz, :])

            def mixes(it):
                # chunk MIXC (keys 454..566) straddles M1: rows [0,MIXR) are
                # keys <= 512 (modality a -> v), the rest modality v -> vc
                am = sb.tile([128, C], BF, name=f"amix_{it}", tag="amix", bufs=2)
                vm = sb.tile([128, C], BF, name=f"vmix_{it}", tag="vmix", bufs=2)
                pcopy(am[:, :], vct[(it, MIXC)][:, :])
                pcopy(am[0:MIXR, :], vt[(it, MIXC)][0:MIXR, :])
                pcopy(vm[:, :], vt[(it, MIXC)][:, :])
                pcopy(vm[0:MIXR, :], vct[(it, MIXC)][0:MIXR, :])
                mix[it] = (am, vm)

            def bank_rotator():
                """Rotating 1-bank psum slots carved from the attention tags
                (sc/av/den/proj) - only valid in the standalone projection
                and tail phases where attention psum is idle. 8-deep rotation
                keeps every group's WAR far behind."""
                views = []

                def get():
                    if not views:
                        t = ps.tile([128, 2048], F32, name="scpp", tag="sc",
                                    bufs=1)
                        a = ps.tile([128, 1024], F32, name="avpp", tag="av",
                                    bufs=1)
                        d = ps.tile([128, 512], F32, name="denpp", tag="den",
                                    bufs=1)
                        q = ps.tile([128, 512], F32, name="pp", tag="proj",
                                    bufs=1)
                        views.extend([t[:, 0:512], t[:, 512:1024],
                                      t[:, 1024:1536], t[:, 1536:2048],
                                      a[:, 0:512], a[:, 512:1024], d, q])
                    return views.pop(0)
                return get

            def filler_units(it):
                # per attn(0)-pair unit lists: pair p carries item1's q/k for
                # pair p (buffer-slot WAR resolves at pair p-1) + v/vc share
                def u(fn, *a):
                    return lambda: fn(*a, ps.tile([128, 512], F32, name="pp",
                                                  tag="proj", bufs=1))
                per_pair = [[] for _ in range(NPAIR)]
                for p in range(NPAIR):
                    for wn in ("wq", "wk"):
                        for qs, qw in QSEG:
                            per_pair[p].append(u(qk_seg, it, p, wn, qs, qw))
                vs = []
                for c in (MIXC, 0, 1, 2, 3, 5, 6, 7):
                    for wn in ("wv", "wvc"):
                        for cs, cw in VSEG:
                            vs.append(u(v_seg, it, c, wn, cs, cw))
                    if c == MIXC:
                        vs.append(lambda it=it: mixes(it))
                nv = len(vs)
                for p in range(NPAIR):
                    per_pair[p] += vs[nv * p // NPAIR: nv * (p + 1) // NPAIR]
                return per_pair

            def outproj_units(it, eng):
                def u(fn, *a):
                    return lambda: fn(*a, ps.tile([128, 512], F32, name="pp",
                                                  tag="proj", bufs=1), eng)
                return [u(op_seg, it, c, cs, cw)
                        for c in range(NCH) for cs, cw in OSEG]

            # ---- attention for one (item, pair); pops fillers each chunk ----
            def attn_pair(it, p, fillers, pops=2):
                q_, k_ = qT[(it, p)], kT[(it, p)]
                # accumulators are pre-zeroed and every matmul uses
                # start=False: correct whether PSUM start-zeroing is
                # bank-wide or per-partition (semantics differ between the
                # simulator model and the HW docs)
                av = ps.tile([128, 1024], F32, name="av", tag="av", bufs=1)
                den = ps.tile([128, 512], F32, name="den", tag="den", bufs=1)
                nc.vector.memset(av[:, :], 0.0)
                nc.vector.memset(den[:, :], 0.0)

                def avden(c, ee, eo_):
                    # AV: 4-way col tiling (128x32). av rows 0:64 = head e,
                    # 64:128 = head o; cols 0:512 = queries 0:512 (mod a),
                    # col 512 = query 512 (mod a), cols 513:907 = queries
                    # 512:906 with mod-v values (col 513 is discarded)
                    sp = (c == NCH - 1)
                    ksz = KCH[c][1]
                    va = mix[it][0] if c == MIXC else (vt[(it, c)] if c < MIXC else vct[(it, c)])
                    vv = mix[it][1] if c == MIXC else (vct[(it, c)] if c < MIXC else vt[(it, c)])
                    for ho, et in ((0, ee), (1, eo_)):
                        for dh in (0, 1):
                            m0 = ho * 64 + dh * 32
                            wc = p * 128 + m0
                            mm(av[m0:m0 + 32, 0:512], lhsT=va[0:ksz, wc:wc + 32],
                               rhs=et[0:ksz, 0:512], start=False, stop=sp,
                               tile_position=(0, m0), skip_group_check=True)
                            mm(av[m0:m0 + 32, 513:907], lhsT=vv[0:ksz, wc:wc + 32],
                               rhs=et[0:ksz, 512:906], start=False, stop=False,
                               tile_position=(0, m0), skip_group_check=True)
                            mm(av[m0:m0 + 32, 512:513], lhsT=va[0:ksz, wc:wc + 32],
                               rhs=et[0:ksz, 512:513], start=False, stop=sp,
                               tile_position=(0, m0), skip_group_check=True)
                    # denominators: 4 ones-column accumulators (rows 0/32/64/96)
                    mm(den[0:1, 0:512], lhsT=ones_sb[0:ksz, 0:1],
                       rhs=ee[0:ksz, 0:512], start=False, stop=sp,
                       tile_position=(0, 0), skip_group_check=True)
                    mm(den[32:33, 0:394], lhsT=ones_sb[0:ksz, 0:1],
                       rhs=ee[0:ksz, 512:906], start=False, stop=sp,
                       tile_position=(0, 32), skip_group_check=True)
                    mm(den[64:65, 0:512], lhsT=ones_sb[0:ksz, 0:1],
                       rhs=eo_[0:ksz, 0:512], start=False, stop=sp,
                       tile_position=(0, 64), skip_group_check=True)
                    mm(den[96:97, 0:394], lhsT=ones_sb[0:ksz, 0:1],
                       rhs=eo_[0:ksz, 512:906], start=False, stop=sp,
                       tile_position=(0, 96), skip_group_check=True)

                # chunk loop is software-pipelined one deep: AV/den for chunk
                # c-1 are emitted after the scores+exp of chunk c so the PE
                # never sits behind the exp latency; filler units go between
                # attention groups so their psum WARs hide under real work
                prev = None
                skip = 2 if (it == 0 and p == 0) else 0
                for c, (ks, ksz) in enumerate(KCH):
                    # scores: e/o heads row-tiled (64x128), separate bank pairs
                    sc = ps.tile([128, 2048], F32, name="sc", tag="sc", bufs=1)
                    for qs, qw in QP:
                        mm(sc[0:ksz, qs:qs + qw],
                           lhsT=k_[0:64, ks:ks + ksz], rhs=q_[0:64, qs:qs + qw],
                           start=True, stop=True)
                        mm(sc[0:ksz, 1024 + qs:1024 + qs + qw],
                           lhsT=k_[64:128, ks:ks + ksz], rhs=q_[64:128, qs:qs + qw],
                           start=True, stop=True)
                    # per-head exps: the e-head banks free while the o-head
                    # exp still runs, letting the next chunk's e-scores start
                    ee = sb.tile([128, 1024], BF, name="ee", tag="exp", bufs=6)
                    eo_ = sb.tile([128, 1024], BF, name="eo", tag="exp", bufs=6)
                    nc.scalar.activation(ee[0:ksz, 0:906], sc[0:ksz, 0:906],
                                         AF.Exp, scale=SCALE)
                    nc.scalar.activation(eo_[0:ksz, 0:906], sc[0:ksz, 1024:1930],
                                         AF.Exp, scale=SCALE)
                    if fillers and c >= skip:
                        fillers.pop(0)()
                    if prev is not None:
                        avden(*prev)
                    prev = (c, ee, eo_)
                    if pops > 1 and fillers and c >= skip:
                        fillers.pop(0)()
                avden(*prev)

                # drain the AV accumulator to SBUF so the next pair's AV can
                # start without waiting on this pair's softmax chain
                avf = sb.tile([128, 1024], F32, name="avf", tag="avf", bufs=2)
                nc.scalar.copy(avf[:, 0:907], av[:, 0:907])

                # ---- softmax division ----
                rc = sb.tile([128, 512], F32, name="rc", tag="rc", bufs=1)
                nc.vector.reciprocal_approx_fast(rc[0:97, 0:512], den[0:97, 0:512])
                # partition_broadcast reads physical partition 0; relocate the
                # three off-zero reciprocal rows there first
                rl = sb.tile([128, 1300], F32, name="rl", tag="rl", bufs=1)
                nc.sync.dma_start(rl[0:1, 0:394], rc[32:33, 0:394])
                nc.sync.dma_start(rl[0:1, 394:906], rc[64:65, 0:512])
                nc.sync.dma_start(rl[0:1, 906:1300], rc[96:97, 0:394])
                bce = sb.tile([128, N], F32, name="bce", tag="bc", bufs=2)
                bco = sb.tile([128, N], F32, name="bco", tag="bc", bufs=2)
                nc.gpsimd.partition_broadcast(bce[:, 0:512], rc[0:1, 0:512])
                nc.gpsimd.partition_broadcast(bce[:, 512:906], rl[0:1, 0:394])
                nc.gpsimd.partition_broadcast(bco[:, 0:512], rl[0:1, 394:906])
                nc.gpsimd.partition_broadcast(bco[:, 512:906], rl[0:1, 906:1300])
                ot = sb.tile([128, N], BF, name=f"oT_{it}_{p}", tag="oT", bufs=9)
                for rows, bc in ((slice(0, 64), bce), (slice(64, 128), bco)):
                    nc.vector.tensor_mul(ot[rows, 0:513], avf[rows, 0:513],
                                         bc[rows, 0:513])
                    nc.vector.tensor_mul(ot[rows, 513:906], avf[rows, 514:907],
                                         bc[rows, 513:906])
                oT[(it, p)] = ot

            # ================= emission =================
            get_pp = bank_rotator()
            for p in range(NPAIR):
                for wn in ("wq", "wk"):
                    for qs, qw in QSEG:
                        qk_seg(0, p, wn, qs, qw, get_pp())
            for c in (MIXC, 0, 1, 2, 3, 5, 6, 7):
                for wn in ("wv", "wvc"):
                    for cs, cw in VSEG:
                        v_seg(0, c, wn, cs, cw, get_pp())
                if c == MIXC:
                    mixes(0)

            load_xT(1)

            fill1 = filler_units(1)
            for p in range(NPAIR):
                fl = fill1[p]
                attn_pair(0, p, fl, pops=2)
                for f in fl:
                    f()

            fill2 = outproj_units(0, nc.sync)
            for p in range(NPAIR):
                attn_pair(1, p, fill2, pops=1)
            for f in fill2:
                f()
            # tail: item 1's output projection on rotating psum banks and
            # rotating DGE queues (everything else is idle by now)
            get_pp = bank_rotator()
            engs = [nc.sync, nc.scalar, nc.gpsimd]
            for c in range(NCH):
                for cs, cw in OSEG:
                    op_seg(1, c, cs, cw, get_pp(), engs[c % 3])

    nc.compile()
    return nc


def _get_built():
    global _BUILT
    if _BUILT is None:
        _BUILT = _build()
    return _BUILT


def kernel(x, Wq, Wk, Wv, Wvc, Wp, bp):
    global LAST_RESULTS
    from concourse.bass_utils import run_bass_kernel_spmd

    x = np.asarray(x, dtype=np.float32)
    bf = ml_dtypes.bfloat16
    xT = np.ascontiguousarray(x.transpose(0, 2, 1)).astype(bf)      # (B, C, N)
    ws = {
        "wq": np.asarray(Wq, dtype=np.float32).astype(bf),
        "wk": np.asarray(Wk, dtype=np.float32).astype(bf),
        "wv": np.asarray(Wv, dtype=np.float32).astype(bf),
        "wvc": np.asarray(Wvc, dtype=np.float32).astype(bf),
        "wp": np.asarray(Wp, dtype=np.float32).astype(bf),
    }
    bias = np.ascontiguousarray(
        np.broadcast_to(np.asarray(bp, dtype=np.float32), (128, C))
    )

    if TRACE:
        _install_trace_shim()

    nc = _get_built()
    in_maps = []
    for i in range(N_CORES):
        m = {"xT": np.ascontiguousarray(xT[i * BPC:(i + 1) * BPC]), "bias": bias}
        m.update(ws)
        in_maps.append(m)

    res = run_bass_kernel_spmd(nc, in_maps, list(range(N_CORES)), trace=TRACE,
                               stitch_traces=False)
    LAST_RESULTS = res
    out = np.concatenate([res.results[i]["out"] for i in range(N_CORES)],
                     axis=0).astype(np.float32)
    return out


# revision 16
# speedup vs baseline: 1.6614x; 1.0194x over previous
"""Multi-modality double-value attention on 8 TRN2 NeuronCores.

Sharding: data-parallel over batch (16 items -> 2 per core). Each core runs
the full attention block for its 2 items; weights are replicated. No
collectives. Host pre-transposes x to x^T and casts inputs to bf16; compute
is bf16 with fp32 PSUM accumulation; output is fp32.

v2: PE-array tiling + software pipelining.
 - scores: 2-way row tiling (64x128 mode) - both heads of a pair run
   concurrently on disjoint PE row groups (K=64 each, no zero padding).
 - AV + softmax denominators: 4-way column tiling (128x32 mode) - the two
   heads' value matmuls (2 x M=32 each) and 4 ones-column denominator
   accumulators share the array.
 - exp: one Scalar-engine activation per (pair, key-chunk) covering both
   heads' scores (reads 4 PSUM banks in a single [ksz, 1930] sweep).
 - key chunks are uniform 113/114 so every matmul keeps the same tile size
   (round_up -> 128) - no PE mode changes from ragged tails.
 - item 1's projections are emitted as filler work inside item 0's
   scalar-bound attention loop; item 0's output projection fills item 1's.
"""

import numpy as np
import ml_dtypes

B, N, C = 16, 906, 768
H = 12
D = 64
M1 = 513
N_CORES = 8
BPC = B // N_CORES          # batch items per core
KC = C // 128               # 6 contraction chunks over C
NPAIR = H // 2              # 6 head pairs
NCH = 8                     # key chunks (uniform 113/114)
_kszs = [114, 114] + [113] * 6
_kst = [sum(_kszs[:i]) for i in range(NCH)]
KCH = list(zip(_kst, _kszs))            # key chunks
MIXC = 4                                 # chunk containing key M1-1=512
MIXR = 512 - _kst[MIXC] + 1              # rows [0,MIXR) of chunk 4 are keys <= 512
QCH = [(i * 128, min(128, N - i * 128)) for i in range(NCH)]  # out-proj row chunks (128 rows -> out DMAs fan across all 16 queues)
QP = [(0, 512), (512, N - 512)]          # query column blocks (A, B)
CPASS = [(0, 512), (512, C - 512)]       # column passes over C
SCALE = D ** -0.5

TRACE = False          # set by test.py to capture a HW profile
LAST_RESULTS = None    # BassKernelResults of the most recent run

_BUILT = None


def _install_trace_shim():
    """The image's antenv lacks axon_hooks; recreate it so trace=True works."""
    import sys, types
    if "antenv.axon_hooks" in sys.modules:
        return
    mod = types.ModuleType("antenv.axon_hooks")
    mod._hook = None
    mod.set_axon_ntff_profile_hook = lambda h: setattr(mod, "_hook", h)
    mod.get_axon_ntff_profile_hook = lambda: mod._hook
    sys.modules["antenv.axon_hooks"] = mod
    import antenv
    antenv.axon_hooks = mod
    from trn_agent_boot.trn_boot import _ntff_profile_via_ctypes
    mod.set_axon_ntff_profile_hook(_ntff_profile_via_ctypes("/opt/axon/libaxon_pjrt.so"))


def _build():
    import concourse.tile as tile
    from concourse import bacc, mybir

    BF = mybir.dt.bfloat16
    F32 = mybir.dt.float32
    AF = mybir.ActivationFunctionType

    nc = bacc.Bacc("TRN2", target_bir_lowering=False, debug=False, num_devices=N_CORES)

    xT_d = nc.dram_tensor("xT", [BPC, C, N], BF, kind="ExternalInput").ap()
    w_d = {
        wn: nc.dram_tensor(wn, [C, C], BF, kind="ExternalInput").ap()
        for wn in ("wq", "wk", "wv", "wvc", "wp")
    }
    bias_d = nc.dram_tensor("bias", [128, C], F32, kind="ExternalInput").ap()
    out_d = nc.dram_tensor("out", [BPC, N, C], BF, kind="ExternalOutput").ap()

    with tile.TileContext(nc) as tc:
        from contextlib import ExitStack
        from concourse import library_config

        with ExitStack() as ctx:
            wpool = ctx.enter_context(tc.tile_pool(name="wpool", bufs=1))
            sb = ctx.enter_context(tc.tile_pool(name="sb", bufs=1))
            ps = ctx.enter_context(tc.tile_pool(name="ps", bufs=1, space="PSUM"))

            nc.gpsimd.load_library(library_config.attn)

            mm = nc.tensor.matmul
            pcopy = nc.vector.tensor_copy

            # ---- constants: weights + bias + ones column ----
            w_sb = {}

            def load_w(wn):
                tiles = []
                for kc in range(KC):
                    t = wpool.tile([128, C], BF, name=f"{wn}_{kc}", tag=f"{wn}_{kc}")
                    nc.sync.dma_start(t[:], w_d[wn][kc * 128:(kc + 1) * 128, :])
                    tiles.append(t)
                w_sb[wn] = tiles

            load_w("wq")

            # x^T item 0 right after wq (they feed the first projections);
            # item 1 is DMA'd after the item-0 projections are emitted
            xT = {}

            def load_xT(it):
                for kc in range(KC):
                    t = sb.tile([128, N], BF, name=f"xT_{it}_{kc}", tag="xT", bufs=8)
                    nc.sync.dma_start(t[:], xT_d[it, kc * 128:(kc + 1) * 128, :])
                    xT[(it, kc)] = t

            load_xT(0)
            load_w("wk")
            load_w("wv")
            load_w("wvc")
            load_w("wp")
            bias_sb = wpool.tile([128, C], F32, name="bias_sb", tag="bias_sb")
            nc.sync.dma_start(bias_sb[:], bias_d[:])
            ones_sb = wpool.tile([128, 1], BF, name="ones_sb", tag="ones_sb")
            nc.vector.memset(ones_sb[:, :], 1.0)
            warm = wpool.tile([128, 1], F32, name="warm", tag="warm")
            nc.scalar.activation(warm[0:1, 0:1], ones_sb[0:1, 0:1], AF.Exp)

            qT, kT, vt, vct, mix, oT = {}, {}, {}, {}, {}, {}

            # ---- segment emitters: each emits ONE 6-matmul psum-bank
            # group + its drain, so consecutive segments in different banks
            # pipeline (a new group in a bank must wait for the previous
            # group's drain-read of that bank)
            QSEG = [(0, 256), (256, 256), (512, 394)]
            VSEG = [(0, 256), (256, 256), (512, 256)]
            OSEG = [(0, 256), (256, 256), (512, 256)]
            obm = {}

            def qk_seg(it, p, wn, qs, qw, pp, drain=None):
                dst_map = qT if wn == "wq" else kT
                if qs == 0:
                    dst_map[(it, p)] = sb.tile(
                        [128, N], BF, name=f"{wn[1]}T_{it}_{p}",
                        tag="qT" if wn == "wq" else "kT", bufs=7)
                dst = dst_map[(it, p)]
                for kc in range(KC):
                    mm(pp[:, 0:qw],
                       lhsT=w_sb[wn][kc][:, p * 128:(p + 1) * 128],
                       rhs=xT[(it, kc)][:, qs:qs + qw],
                       start=(kc == 0), stop=(kc == KC - 1))
                (drain or pcopy)(dst[:, qs:qs + qw], pp[:, 0:qw])

            def v_seg(it, c, wn, cs, cw, pp):
                dst_map = vt if wn == "wv" else vct
                ts, tsz = KCH[c]
                if cs == 0:
                    dst = sb.tile([128, C], BF, name=f"{wn[1:]}_{it}_{c}",
                                  tag="v" if wn == "wv" else "vc", bufs=16)
                    # AV stationary loads may touch all 128 partitions; keep
                    # the unwritten tail rows finite
                    nc.vector.memset(dst[96:128, :], 0.0)
                    dst_map[(it, c)] = dst
                dst = dst_map[(it, c)]
                for kc in range(KC):
                    mm(pp[0:tsz, 0:cw],
                       lhsT=xT[(it, kc)][:, ts:ts + tsz],
                       rhs=w_sb[wn][kc][:, cs:cs + cw],
                       start=(kc == 0), stop=(kc == KC - 1))
                pcopy(dst[0:tsz, cs:cs + cw], pp[0:tsz, 0:cw])

            def op_seg(it, c, cs, cw, pp, eng):
                ts, tsz = QCH[c]
                if cs == 0:
                    obm[(it, c)] = sb.tile([128, C], BF, name="ob", tag="ob",
                                           bufs=3)
                ob = obm[(it, c)]
                for kp in range(NPAIR):
                    mm(pp[0:tsz, 0:cw],
                       lhsT=oT[(it, kp)][:, ts:ts + tsz],
                       rhs=w_sb["wp"][kp][:, cs:cs + cw],
                       start=(kp == 0), stop=(kp == NPAIR - 1))
                nc.vector.tensor_add(ob[0:tsz, cs:cs + cw], pp[0:tsz, 0:cw],
                                     bias_sb[0:tsz, cs:cs + cw])
                if cs + cw == C:
                    eng.dma_start(out_d[it, ts:ts + tsz, :], ob[0:tsz, :])

            def mixes(it):
                # chunk MIXC (keys 454..566) straddles M1: rows [0,MIXR) are
                # keys <= 512 (modality a -> v), the rest modality v -> vc
                am = sb.tile([128, C], BF, name=f"amix_{it}", tag="amix", bufs=2)
                vm = sb.tile([128, C], BF, name=f"vmix_{it}", tag="vmix", bufs=2)
                pcopy(am[:, :], vct[(it, MIXC)][:, :])
                pcopy(am[0:MIXR, :], vt[(it, MIXC)][0:MIXR, :])
                pcopy(vm[:, :], vt[(it, MIXC)][:, :])
                pcopy(vm[0:MIXR, :], vct[(it, MIXC)][0:MIXR, :])
                mix[it] = (am, vm)

            def bank_rotator():
                """Rotating 1-bank psum slots carved from the attention tags
                (sc/av/den/proj) - only valid in the standalone projection
                and tail phases where attention psum is idle. 8-deep rotation
                keeps every group's WAR far behind."""
                views = []

                def get():
                    if not views:
                        t = ps.tile([128, 2048], F32, name="scpp", tag="sc",
                                    bufs=1)
                        a = ps.tile([128, 1024], F32, name="avpp", tag="av",
                                    bufs=1)
                        d = ps.tile([128, 512], F32, name="denpp", tag="den",
                                    bufs=1)
                        q = ps.tile([128, 512], F32, name="pp", tag="proj",
                                    bufs=1)
                        views.extend([t[:, 0:512], t[:, 512:1024],
                                      t[:, 1024:1536], t[:, 1536:2048],
                                      a[:, 0:512], a[:, 512:1024], d, q])
                    return views.pop(0)
                return get

            def filler_units(it):
                # per attn(0)-pair unit lists: pair p carries item1's q/k for
                # pair p (buffer-slot WAR resolves at pair p-1) + v/vc share
                def u(fn, *a):
                    return lambda: fn(*a, ps.tile([128, 512], F32, name="pp",
                                                  tag="proj", bufs=1))
                per_pair = [[] for _ in range(NPAIR)]
                for p in range(NPAIR):
                    for wn in ("wq", "wk"):
                        for qs, qw in QSEG:
                            per_pair[p].append(u(qk_seg, it, p, wn, qs, qw))
                vs = []
                for c in (MIXC, 0, 1, 2, 3, 5, 6, 7):
                    for wn in ("wv", "wvc"):
                        for cs, cw in VSEG:
                            vs.append(u(v_seg, it, c, wn, cs, cw))
                    if c == MIXC:
                        vs.append(lambda it=it: mixes(it))
                nv = len(vs)
                for p in range(NPAIR):
                    per_pair[p] += vs[nv * p // NPAIR: nv * (p + 1) // NPAIR]
                return per_pair

            def outproj_units(it, eng):
                def u(fn, *a):
                    return lambda: fn(*a, ps.tile([128, 512], F32, name="pp",
                                                  tag="proj", bufs=1), eng)
                return [u(op_seg, it, c, cs, cw)
                        for c in range(NCH) for cs, cw in OSEG]

            # ---- attention for one (item, pair); pops fillers each chunk ----
            def attn_pair(it, p, fillers, pops=2):
                q_, k_ = qT[(it, p)], kT[(it, p)]
                # accumulators are pre-zeroed and every matmul uses
                # start=False: correct whether PSUM start-zeroing is
                # bank-wide or per-partition (semantics differ between the
                # simulator model and the HW docs)
                av = ps.tile([128, 1024], F32, name="av", tag="av", bufs=1)
                den = ps.tile([128, 512], F32, name="den", tag="den", bufs=1)
                nc.vector.memset(av[:, :], 0.0)
                nc.vector.memset(den[:, :], 0.0)

                def avden(c, ee, eo_):
                    # AV: 4-way col tiling (128x32). av rows 0:64 = head e,
                    # 64:128 = head o; cols 0:512 = queries 0:512 (mod a),
                    # col 512 = query 512 (mod a), cols 513:907 = queries
                    # 512:906 with mod-v values (col 513 is discarded)
                    sp = (c == NCH - 1)
                    ksz = KCH[c][1]
                    va = mix[it][0] if c == MIXC else (vt[(it, c)] if c < MIXC else vct[(it, c)])
                    vv = mix[it][1] if c == MIXC else (vct[(it, c)] if c < MIXC else vt[(it, c)])
                    for ho, et in ((0, ee), (1, eo_)):
                        for dh in (0, 1):
                            m0 = ho * 64 + dh * 32
                            wc = p * 128 + m0
                            mm(av[m0:m0 + 32, 0:512], lhsT=va[0:ksz, wc:wc + 32],
                               rhs=et[0:ksz, 0:512], start=False, stop=sp,
                               tile_position=(0, m0), skip_group_check=True)
                            mm(av[m0:m0 + 32, 513:907], lhsT=vv[0:ksz, wc:wc + 32],
                               rhs=et[0:ksz, 512:906], start=False, stop=False,
                               tile_position=(0, m0), skip_group_check=True)
                            mm(av[m0:m0 + 32, 512:513], lhsT=va[0:ksz, wc:wc + 32],
                               rhs=et[0:ksz, 512:513], start=False, stop=sp,
                               tile_position=(0, m0), skip_group_check=True)
                    # denominators: 4 ones-column accumulators (rows 0/32/64/96)
                    mm(den[0:1, 0:512], lhsT=ones_sb[0:ksz, 0:1],
                       rhs=ee[0:ksz, 0:512], start=False, stop=sp,
                       tile_position=(0, 0), skip_group_check=True)
                    mm(den[32:33, 0:394], lhsT=ones_sb[0:ksz, 0:1],
                       rhs=ee[0:ksz, 512:906], start=False, stop=sp,
                       tile_position=(0, 32), skip_group_check=True)
                    mm(den[64:65, 0:512], lhsT=ones_sb[0:ksz, 0:1],
                       rhs=eo_[0:ksz, 0:512], start=False, stop=sp,
                       tile_position=(0, 64), skip_group_check=True)
                    mm(den[96:97, 0:394], lhsT=ones_sb[0:ksz, 0:1],
                       rhs=eo_[0:ksz, 512:906], start=False, stop=sp,
                       tile_position=(0, 96), skip_group_check=True)

                # chunk loop is software-pipelined one deep: AV/den for chunk
                # c-1 are emitted after the scores+exp of chunk c so the PE
                # never sits behind the exp latency; filler units go between
                # attention groups so their psum WARs hide under real work
                prev = None
                skip = 1 if (it == 0 and p == 0) else 0
                for c, (ks, ksz) in enumerate(KCH):
                    # scores: e/o heads row-tiled (64x128), separate bank pairs
                    sc = ps.tile([128, 2048], F32, name="sc", tag="sc", bufs=1)
                    for qs, qw in QP:
                        mm(sc[0:ksz, qs:qs + qw],
                           lhsT=k_[0:64, ks:ks + ksz], rhs=q_[0:64, qs:qs + qw],
                           start=True, stop=True)
                        mm(sc[0:ksz, 1024 + qs:1024 + qs + qw],
                           lhsT=k_[64:128, ks:ks + ksz], rhs=q_[64:128, qs:qs + qw],
                           start=True, stop=True)
                    # per-head exps: the e-head banks free while the o-head
                    # exp still runs, letting the next chunk's e-scores start
                    ee = sb.tile([128, 1024], BF, name="ee", tag="exp", bufs=6)
                    eo_ = sb.tile([128, 1024], BF, name="eo", tag="exp", bufs=6)
                    nc.scalar.activation(ee[0:ksz, 0:906], sc[0:ksz, 0:906],
                                         AF.Exp, scale=SCALE)
                    nc.scalar.activation(eo_[0:ksz, 0:906], sc[0:ksz, 1024:1930],
                                         AF.Exp, scale=SCALE)
                    if fillers and c >= skip:
                        fillers.pop(0)()
                    if prev is not None:
                        avden(*prev)
                    prev = (c, ee, eo_)
                    if pops > 1 and fillers and c >= skip:
                        fillers.pop(0)()
                avden(*prev)

                # drain the AV accumulator to SBUF so the next pair's AV can
                # start without waiting on this pair's softmax chain
                avf = sb.tile([128, 1024], F32, name="avf", tag="avf", bufs=2)
                nc.scalar.copy(avf[:, 0:907], av[:, 0:907])

                # ---- softmax division ----
                rc = sb.tile([128, 512], F32, name="rc", tag="rc", bufs=1)
                nc.vector.reciprocal_approx_fast(rc[0:97, 0:512], den[0:97, 0:512])
                # partition_broadcast reads physical partition 0; relocate the
                # three off-zero reciprocal rows there first
                rl = sb.tile([128, 1300], F32, name="rl", tag="rl", bufs=1)
                nc.sync.dma_start(rl[0:1, 0:394], rc[32:33, 0:394])
                nc.sync.dma_start(rl[0:1, 394:906], rc[64:65, 0:512])
                nc.sync.dma_start(rl[0:1, 906:1300], rc[96:97, 0:394])
                bce = sb.tile([128, N], F32, name="bce", tag="bc", bufs=2)
                bco = sb.tile([128, N], F32, name="bco", tag="bc", bufs=2)
                nc.gpsimd.partition_broadcast(bce[:, 0:512], rc[0:1, 0:512])
                nc.gpsimd.partition_broadcast(bce[:, 512:906], rl[0:1, 0:394])
                nc.gpsimd.partition_broadcast(bco[:, 0:512], rl[0:1, 394:906])
                nc.gpsimd.partition_broadcast(bco[:, 512:906], rl[0:1, 906:1300])
                ot = sb.tile([128, N], BF, name=f"oT_{it}_{p}", tag="oT", bufs=9)
                for rows, bc in ((slice(0, 64), bce), (slice(64, 128), bco)):
                    nc.vector.tensor_mul(ot[rows, 0:513], avf[rows, 0:513],
                                         bc[rows, 0:513])
                    nc.vector.tensor_mul(ot[rows, 513:906], avf[rows, 514:907],
                                         bc[rows, 513:906])
                oT[(it, p)] = ot

            # ================= emission =================
            get_pp = bank_rotator()
            for p in range(NPAIR):
                for wn in ("wq", "wk"):
                    for qs, qw in QSEG:
                        qk_seg(0, p, wn, qs, qw, get_pp(), drain=nc.scalar.copy)
            for c in (MIXC, 0, 1, 2, 3, 5, 6, 7):
                for wn in ("wv", "wvc"):
                    for cs, cw in VSEG:
                        v_seg(0, c, wn, cs, cw, get_pp())
                if c == MIXC:
                    mixes(0)

            load_xT(1)

            fill1 = filler_units(1)
            for p in range(NPAIR):
                fl = fill1[p]
                attn_pair(0, p, fl, pops=2)
                for f in fl:
                    f()

            fill2 = outproj_units(0, nc.sync)
            for p in range(NPAIR):
                attn_pair(1, p, fill2, pops=1)
            for f in fill2:
                f()
            # tail: item 1's output projection on rotating psum banks and
            # rotating DGE queues (everything else is idle by now)
            get_pp = bank_rotator()
            engs = [nc.sync, nc.scalar, nc.gpsimd]
            for c in range(NCH):
                for cs, cw in OSEG:
                    op_seg(1, c, cs, cw, get_pp(), engs[c % 3])

    nc.compile()
    return nc


def _get_built():
    global _BUILT
    if _BUILT is None:
        _BUILT = _build()
    return _BUILT


def kernel(x, Wq, Wk, Wv, Wvc, Wp, bp):
    global LAST_RESULTS
    from concourse.bass_utils import run_bass_kernel_spmd

    x = np.asarray(x, dtype=np.float32)
    bf = ml_dtypes.bfloat16
    xT = np.ascontiguousarray(x.transpose(0, 2, 1)).astype(bf)      # (B, C, N)
    ws = {
        "wq": np.asarray(Wq, dtype=np.float32).astype(bf),
        "wk": np.asarray(Wk, dtype=np.float32).astype(bf),
        "wv": np.asarray(Wv, dtype=np.float32).astype(bf),
        "wvc": np.asarray(Wvc, dtype=np.float32).astype(bf),
        "wp": np.asarray(Wp, dtype=np.float32).astype(bf),
    }
    bias = np.ascontiguousarray(
        np.broadcast_to(np.asarray(bp, dtype=np.float32), (128, C))
    )

    if TRACE:
        _install_trace_shim()

    nc = _get_built()
    in_maps = []
    for i in range(N_CORES):
        m = {"xT": np.ascontiguousarray(xT[i * BPC:(i + 1) * BPC]), "bias": bias}
        m.update(ws)
        in_maps.append(m)

    res = run_bass_kernel_spmd(nc, in_maps, list(range(N_CORES)), trace=TRACE,
                               stitch_traces=False)
    LAST_RESULTS = res
    out = np.concatenate([res.results[i]["out"] for i in range(N_CORES)],
                     axis=0).astype(np.float32)
    return out
